# revision 1
# baseline (speedup 1.0000x reference)
"""MLA + DeepSeekMoE block on 8 trn2 NeuronCores (Bass/Tile SPMD).

Token-sharded across 8 cores (512 tokens each, causally balanced stripes).
v2: no collective -- each core recomputes the KV latent (kv256+rope64) for
all 2048 batch tokens from a bf16 copy of x (MLA absorption trick).
Attention operands in bf16 (same PE rate as fp32r, half the SBUF/DVE).
MoE (shared + all 8 routed experts, computed densely) in fp8-e4m3 with
DoubleRow matmuls; expert w2 outputs accumulate across experts in PSUM.
cproj/gate stay fp32r so the gate top-2 selection matches the reference.
"""

import numpy as np
import ml_dtypes

import concourse.bacc as bacc
import concourse.mybir as mybir
from concourse.tile import TileContext
from concourse.bass_utils import run_bass_kernel_spmd

# ---- problem constants ----
D = 1024; H = 8; QLR = 384; KVLR = 256; NOPE = 128; ROPE = 64; VD = 128
E = 8; TOPK = 2; INTER = 512; NSH = 2; B = 2; T = 2048; QKD = NOPE + ROPE
N = B * T
NCORES = 8
NLOC = N // NCORES          # 512 tokens per core
P = 128
EPS = 1e-6
SCALE = 1.0 / np.sqrt(QKD)
NEG = -1e9
QW = 64.0                   # fp8 weight scale
HS = 16.0                   # fp8 h2 / hsh scale

F32 = mybir.dt.float32
F32R = mybir.dt.float32r
BF16 = mybir.dt.bfloat16
FP8 = mybir.dt.float8e4
DR = mybir.MatmulPerfMode.DoubleRow
E4NP = ml_dtypes.float8_e4m3
BFNP = ml_dtypes.bfloat16

# MoE fp8 scale plan:
#   h2f8 = HS*h2;  w1,w3,w2 fp8 x QW
#   g1 psum = HS*QW*g1_true -> silu(scale=1/(HS*QW)) = silu_true
#   tmp = silu*g3 = HS*QW*hsh_true
#   hsh8 = tmp * cwb,  cwb = cw/QW  (sel8 prescaled 1/QW; shared cw=1)
#   acc psum = hsh8 @ w2(xQW) = HS*QW*(cw-weighted moe) -> copy scale 1/(HS*QW)


def _rope_perm():
    return np.concatenate([np.arange(0, ROPE, 2), np.arange(1, ROPE, 2)])


def _core_positions(c):
    j = c % 4
    return np.concatenate([np.arange(j * 256, (j + 1) * 256),
                           np.arange((7 - j) * 256, (8 - j) * 256)])


def _tile_w(w):
    """[K, F] row-major -> [128, K//128, F] partition-major contiguous."""
    K, F = w.shape
    return np.ascontiguousarray(w.reshape(K // P, P, F).transpose(1, 0, 2))


def _f8(a):
    return np.ascontiguousarray(np.asarray(a, np.float32)).astype(E4NP)


# ============================ device program ============================

def build():
    from contextlib import ExitStack as ES
    nc = bacc.Bacc(name="mla_moe_v2")

    # ---- I/O ----
    xT = nc.dram_tensor("xT", [D, NLOC], F32R, kind="ExternalInput")
    xTb = nc.dram_tensor("xTb", [P, 8, NLOC], BF16, kind="ExternalInput")
    xb = nc.dram_tensor("xb", [P, 8, T], BF16, kind="ExternalInput")
    cosbT = nc.dram_tensor("cosbT", [ROPE // 2, T], BF16, kind="ExternalInput")
    sinbT = nc.dram_tensor("sinbT", [ROPE // 2, T], BF16, kind="ExternalInput")
    cosT = nc.dram_tensor("cosT", [ROPE // 2, NLOC], BF16, kind="ExternalInput")
    sinT = nc.dram_tensor("sinT", [ROPE // 2, NLOC], BF16, kind="ExternalInput")
    mask1 = nc.dram_tensor("mask1", [8, P, NLOC], BF16, kind="ExternalInput")
    mask2 = nc.dram_tensor("mask2", [8, P, 256], BF16, kind="ExternalInput")
    ident = nc.dram_tensor("ident", [P, P], F32, kind="ExternalInput")
    identb = nc.dram_tensor("identb", [P, P], BF16, kind="ExternalInput")
    onesbf = nc.dram_tensor("onesbf", [P, 1], BF16, kind="ExternalInput")
    ones1f = nc.dram_tensor("ones1f", [1, P], F32, kind="ExternalInput")
    onesr = nc.dram_tensor("onesr", [P, 1], F32R, kind="ExternalInput")
    latq_w = nc.dram_tensor("latq_w", [P, 8, QLR], BF16, kind="ExternalInput")
    latkv_w = nc.dram_tensor("latkv_w", [P, 8, KVLR + ROPE], BF16, kind="ExternalInput")
    q_up = nc.dram_tensor("q_up", [P, 3, H * QKD], F32R, kind="ExternalInput")
    wkTT = nc.dram_tensor("wkTT", [P, H, KVLR], F32R, kind="ExternalInput")
    wv_w = nc.dram_tensor("wv_w", [P, 2, H * VD], F32R, kind="ExternalInput")
    cproj_w = nc.dram_tensor("cproj_w", [P, 8, D], F32R, kind="ExternalInput")
    gate_w = nc.dram_tensor("gate_w", [P, 8, E], F32R, kind="ExternalInput")
    shw1 = nc.dram_tensor("shw1", [P, 8, INTER * NSH], FP8, kind="ExternalInput")
    shw3 = nc.dram_tensor("shw3", [P, 8, INTER * NSH], FP8, kind="ExternalInput")
    shw2 = nc.dram_tensor("shw2", [P, 8, D], BF16, kind="ExternalInput")
    ew1 = nc.dram_tensor("ew1", [E, P, 8, INTER], FP8, kind="ExternalInput")
    ew3 = nc.dram_tensor("ew3", [E, P, 8, INTER], FP8, kind="ExternalInput")
    ew2 = nc.dram_tensor("ew2", [E, P, 4, D], BF16, kind="ExternalInput")
    sel8 = nc.dram_tensor("sel8", [E, E * P], BF16, kind="ExternalInput")
    out_xT = nc.dram_tensor("out_xT", [D, NLOC], F32R, kind="ExternalOutput")
    dbg_yT = nc.dram_tensor("dbg_yT", [P, H, NLOC], F32R, kind="ExternalOutput")
    dbg_x2 = nc.dram_tensor("dbg_x2", [P, 8, NLOC], F32R, kind="ExternalOutput")
    dbg_kvn = nc.dram_tensor("dbg_kvn", [P, 2, T], F32R, kind="ExternalOutput")
    dbg_krx = nc.dram_tensor("dbg_krx", [ROPE, T], BF16, kind="ExternalOutput")
    dbg_qabs = nc.dram_tensor("dbg_qabs", [P, 2 * H, NLOC], F32R, kind="ExternalOutput")
    dbg_st = nc.dram_tensor("dbg_st", [P, NLOC], F32, kind="ExternalOutput")
    dbg_pt = nc.dram_tensor("dbg_pt", [P, NLOC], F32R, kind="ExternalOutput")

    AL = mybir.AluOpType
    AF = mybir.ActivationFunctionType

    with TileContext(nc) as tc, \
         nc.allow_low_precision(reason="f32r rows / bf16+fp8 activations by design"), \
         tc.tile_pool(name="const", bufs=1) as p_const, \
         tc.tile_pool(name="psc", bufs=2) as p_sc:

        # right-side LIFO stack (open order = reverse close order)
        g_x = ES();  p_x = g_x.enter_context(tc.tile_pool(name="px", bufs=1, side="right"))
        g_yT = ES(); p_yt = g_yT.enter_context(tc.tile_pool(name="pyt", bufs=1, side="right"))
        g_kv = ES(); p_kv = g_kv.enter_context(tc.tile_pool(name="pkv", bufs=1, side="right"))
        g_q = ES();  p_q = g_q.enter_context(tc.tile_pool(name="pq", bufs=1, side="right"))
        g_a = ES()
        p_wA = g_a.enter_context(tc.tile_pool(name="pwA", bufs=1, side="right"))
        p_qu = g_a.enter_context(tc.tile_pool(name="pqu", bufs=1, side="right"))
        p_xb = g_a.enter_context(tc.tile_pool(name="pxb", bufs=2, side="right"))

        # ---- constants ----
        identf = p_const.tile([P, P], F32, tag="identf")
        nc.gpsimd.dma_start(out=identf[:], in_=ident[:])
        identr = p_const.tile([P, P], F32R, tag="identr")
        nc.vector.tensor_copy(out=identr[:], in_=identf[:])
        identbf = p_const.tile([P, P], BF16, tag="identbf")
        nc.gpsimd.dma_start(out=identbf[:], in_=identb[:])
        onesbf_sb = p_const.tile([P, 1], BF16, tag="onesbf")
        nc.gpsimd.dma_start(out=onesbf_sb[:], in_=onesbf[:])
        ones1f_sb = p_const.tile([1, P], F32, tag="ones1f")
        nc.gpsimd.dma_start(out=ones1f_sb[:], in_=ones1f[:])
        onesr_sb = p_const.tile([P, 1], F32R, tag="onesr")
        nc.gpsimd.dma_start(out=onesr_sb[:], in_=onesr[:])
        eps1 = p_const.tile([1, 1], F32, tag="eps1")
        nc.vector.memset(eps1[:], EPS)
        cos_sb = p_const.tile([ROPE // 2, NLOC], BF16, tag="cos")
        nc.gpsimd.dma_start(out=cos_sb[:], in_=cosT[:])
        sin_sb = p_const.tile([ROPE // 2, NLOC], BF16, tag="sin")
        nc.gpsimd.dma_start(out=sin_sb[:], in_=sinT[:])
        cosb_sb = p_const.tile([ROPE // 2, T], BF16, tag="cosb")
        nc.gpsimd.dma_start(out=cosb_sb[:], in_=cosbT[:])
        sinb_sb = p_const.tile([ROPE // 2, T], BF16, tag="sinb")
        nc.gpsimd.dma_start(out=sinb_sb[:], in_=sinbT[:])

        srow2 = p_const.tile([1, NLOC], F32, tag="srow2")

        # ---- persistent activations ----
        x2 = p_x.tile([P, 8, NLOC], F32R, tag="x2")
        h2f8 = p_x.tile([P, 8, NLOC], FP8, tag="h2f8")
        cwT = p_x.tile([E, NLOC], BF16, tag="cwT")
        yT = p_yt.tile([P, H, NLOC], F32R, tag="yT")

        kvn = p_kv.tile([P, 2, T], F32R, tag="kvn")
        krx = p_kv.tile([ROPE, T], BF16, tag="krx")

        qabs = p_q.tile([P, 2 * H, NLOC], F32R, tag="qabs")
        qrope = p_q.tile([ROPE, H, NLOC], BF16, tag="qrope")

        xTb_sb = p_wA.tile([P, 8, NLOC], BF16, tag="xTb")
        nc.sync.dma_start(out=xTb_sb[:], in_=xTb[:])
        latq_sb = p_wA.tile([P, 8, QLR], BF16, tag="latqw")
        nc.sync.dma_start(out=latq_sb[:], in_=latq_w[:])
        latkv_sb = p_wA.tile([P, 8, KVLR + ROPE], BF16, tag="latkvw")
        nc.sync.dma_start(out=latkv_sb[:], in_=latkv_w[:])
        wkTT_sb = p_wA.tile([P, H, KVLR], F32R, tag="wkTT")
        nc.sync.dma_start(out=wkTT_sb[:], in_=wkTT[:])

        def brow(ps_pool, sb_pool, row_ap, n, tag="bcsb", dtype=F32, ptag="bc"):
            bc = ps_pool.tile([P, n], F32, tag=ptag)
            nc.tensor.matmul(bc[:], ones1f_sb[:], row_ap, start=True, stop=True)
            sb = sb_pool.tile([P, n], dtype, tag=tag)
            nc.vector.tensor_copy(out=sb[:], in_=bc[:])
            return sb

        # ================= phase A =================
        with tc.tile_pool(name="actA", bufs=1) as p_actA, \
             tc.tile_pool(name="pr", bufs=1) as p_r, \
             tc.tile_pool(name="pslat", bufs=2, space="PSUM") as ps_lat, \
             tc.tile_pool(name="psrow", bufs=2, space="PSUM") as ps_row, \
             tc.tile_pool(name="psbc", bufs=1, space="PSUM") as ps_bc:

            # ---- A1: local rms1 + q latents (bf16 inputs; scores-only) ----
            ss_ps = ps_row.tile([1, NLOC], F32, tag="ss")
            for ds in range(8):
                xsq = p_sc.tile([P, NLOC], BF16, tag="xsq")
                nc.vector.tensor_mul(out=xsq[:], in0=xTb_sb[:, ds, :], in1=xTb_sb[:, ds, :])
                nc.tensor.matmul(ss_ps[:], onesbf_sb[:], xsq[:],
                                 start=(ds == 0), stop=(ds == 7))
            row = p_sc.tile([1, NLOC], F32, tag="row")
            nc.scalar.activation(out=row[:], in_=ss_ps[:],
                                 func=AF.Sqrt, bias=eps1[:], scale=1.0 / D)
            nc.vector.reciprocal(out=row[:], in_=row[:])
            s1loc = brow(ps_bc, p_sc, row[:], NLOC, tag="bcsb")

            qln = p_actA.tile([P, 3, NLOC], F32R, tag="qln")
            for ft in range(3):
                lp = ps_lat.tile([P, NLOC], F32, tag="lat")
                for ds in range(8):
                    nc.tensor.matmul(lp[:], latq_sb[:, ds, ft * 128:(ft + 1) * 128],
                                     xTb_sb[:, ds, :], start=(ds == 0), stop=(ds == 7))
                nc.vector.tensor_tensor(out=qln[:, ft, :], in0=lp[:],
                                        in1=s1loc[:], op=AL.mult)
            sq_ps = ps_row.tile([1, NLOC], F32, tag="ss")
            for t in range(3):
                xsq = p_sc.tile([P, NLOC], BF16, tag="xsq")
                nc.vector.tensor_mul(out=xsq[:], in0=qln[:, t, :], in1=qln[:, t, :])
                nc.tensor.matmul(sq_ps[:], onesbf_sb[:], xsq[:],
                                 start=(t == 0), stop=(t == 2))
            row = p_sc.tile([1, NLOC], F32, tag="row")
            nc.scalar.activation(out=row[:], in_=sq_ps[:],
                                 func=AF.Sqrt, bias=eps1[:], scale=1.0 / QLR)
            nc.vector.reciprocal(out=row[:], in_=row[:])
            nc.vector.tensor_scalar_mul(out=row[:], in0=row[:],
                                        scalar1=float(SCALE))
            sqb = brow(ps_bc, p_sc, row[:], NLOC, tag="bcsb")
            for t in range(3):
                nc.vector.tensor_tensor(out=qln[:, t, :], in0=qln[:, t, :],
                                        in1=sqb[:], op=AL.mult)

            # ---- A2: batch kv latents, 4 chunks of 512 tokens ----
            for c4 in range(4):
                cs = slice(c4 * 512, (c4 + 1) * 512)
                xbc = p_xb.tile([P, 8, NLOC], BF16, tag="xbc")
                nc.sync.dma_start(out=xbc[:], in_=xb[:, :, cs])
                ssb = ps_row.tile([1, NLOC], F32, tag="ss")
                for ds in range(8):
                    xsq = p_sc.tile([P, NLOC], BF16, tag="xsq")
                    nc.vector.tensor_mul(out=xsq[:], in0=xbc[:, ds, :],
                                         in1=xbc[:, ds, :])
                    nc.tensor.matmul(ssb[:], onesbf_sb[:], xsq[:],
                                     start=(ds == 0), stop=(ds == 7))
                row = p_sc.tile([1, NLOC], F32, tag="row")
                nc.scalar.activation(out=row[:], in_=ssb[:],
                                     func=AF.Sqrt, bias=eps1[:], scale=1.0 / D)
                nc.vector.reciprocal(out=row[:], in_=row[:])
                s1b = brow(ps_bc, p_sc, row[:], NLOC, tag="bcsb")

                kvt = p_r.tile([P, 2, NLOC], F32, tag="kvt")
                for i in range(2):
                    lp = ps_lat.tile([P, NLOC], F32, tag="lat")
                    for ds in range(8):
                        nc.tensor.matmul(lp[:], latkv_sb[:, ds, i * 128:(i + 1) * 128],
                                         xbc[:, ds, :], start=(ds == 0), stop=(ds == 7))
                    nc.vector.tensor_tensor(out=kvt[:, i, :], in0=lp[:],
                                            in1=s1b[:], op=AL.mult)
                kss = ps_row.tile([1, NLOC], F32, tag="ss")
                for i in range(2):
                    xsq = p_sc.tile([P, NLOC], BF16, tag="xsq")
                    nc.vector.tensor_mul(out=xsq[:], in0=kvt[:, i, :], in1=kvt[:, i, :])
                    nc.tensor.matmul(kss[:], onesbf_sb[:], xsq[:],
                                     start=(i == 0), stop=(i == 1))
                row = p_sc.tile([1, NLOC], F32, tag="row")
                nc.scalar.activation(out=row[:], in_=kss[:],
                                     func=AF.Sqrt, bias=eps1[:], scale=1.0 / KVLR)
                nc.vector.reciprocal(out=row[:], in_=row[:])
                skvb = brow(ps_bc, p_sc, row[:], NLOC, tag="bcsb")
                for i in range(2):
                    nc.vector.tensor_tensor(out=kvn[:, i, cs], in0=kvt[:, i, :],
                                            in1=skvb[:], op=AL.mult)
                # rope cols x s1, write bf16
                lp = ps_lat.tile([P, NLOC], F32, tag="lat")
                for ds in range(8):
                    nc.tensor.matmul(lp[:ROPE], latkv_sb[:, ds, KVLR:KVLR + ROPE],
                                     xbc[:, ds, :], start=(ds == 0), stop=(ds == 7))
                cs1 = p_r.tile([ROPE // 2, NLOC], F32, tag="cs1")
                ss1 = p_r.tile([ROPE // 2, NLOC], F32, tag="ss1")
                nc.vector.tensor_mul(out=cs1[:], in0=cosb_sb[:, cs], in1=s1b[0:32, :])
                nc.vector.tensor_mul(out=ss1[:], in0=sinb_sb[:, cs], in1=s1b[0:32, :])
                od = p_r.tile([ROPE // 2, NLOC], F32, tag="od")
                nc.vector.tensor_copy(out=od[:], in_=lp[32:64])
                t1 = p_r.tile([ROPE // 2, NLOC], F32, tag="t1")
                t2 = p_r.tile([ROPE // 2, NLOC], F32, tag="t2")
                nc.vector.tensor_mul(out=t1[:], in0=lp[0:32], in1=cs1[:])
                nc.vector.tensor_mul(out=t2[:], in0=od[:], in1=ss1[:])
                nc.vector.tensor_sub(out=krx[0:32, cs], in0=t1[:], in1=t2[:])
                nc.vector.tensor_mul(out=t1[:], in0=lp[0:32], in1=ss1[:])
                nc.vector.tensor_mul(out=t2[:], in0=od[:], in1=cs1[:])
                nc.vector.tensor_add(out=krx[32:64, cs], in0=t1[:], in1=t2[:])

            # ---- A3: q per head (q_up streamed in halves) ----
            with tc.tile_pool(name="psqp", bufs=3, space="PSUM") as ps_qp:
                for hg in range(2):
                    qup_sb = p_qu.tile([P, 3, 4 * QKD], F32R, tag="qup")
                    nc.sync.dma_start(out=qup_sb[:],
                                      in_=q_up[:, :, hg * 4 * QKD:(hg + 1) * 4 * QKD])
                    for hh in range(4):
                        h = hg * 4 + hh
                        qn_ps = ps_qp.tile([P, NLOC], F32, tag="qp")
                        for t in range(3):
                            nc.tensor.matmul(qn_ps[:],
                                             qup_sb[:, t, hh * QKD:hh * QKD + NOPE],
                                             qln[:, t, :], start=(t == 0), stop=(t == 2))
                        qn_sb = p_sc.tile([P, NLOC], F32R, tag="qnsb")
                        nc.vector.tensor_copy(out=qn_sb[:], in_=qn_ps[:])
                        for i in range(2):
                            qa_ps = ps_qp.tile([P, NLOC], F32, tag="qp")
                            nc.tensor.matmul(qa_ps[:], wkTT_sb[:, h, i * 128:(i + 1) * 128],
                                             qn_sb[:], start=True, stop=True)
                            nc.vector.tensor_copy(out=qabs[:, 2 * h + i, :], in_=qa_ps[:])
                        qr_ps = ps_qp.tile([P, NLOC], F32, tag="qp")
                        for t in range(3):
                            nc.tensor.matmul(qr_ps[:ROPE],
                                             qup_sb[:, t, hh * QKD + NOPE:(hh + 1) * QKD],
                                             qln[:, t, :], start=(t == 0), stop=(t == 2))
                        od = p_r.tile([ROPE // 2, NLOC], F32, tag="od")
                        nc.vector.tensor_copy(out=od[:], in_=qr_ps[32:64])
                        t1 = p_r.tile([ROPE // 2, NLOC], F32, tag="t1")
                        t2 = p_r.tile([ROPE // 2, NLOC], F32, tag="t2")
                        nc.vector.tensor_mul(out=t1[:], in0=qr_ps[0:32], in1=cos_sb[:])
                        nc.vector.tensor_mul(out=t2[:], in0=od[:], in1=sin_sb[:])
                        nc.vector.tensor_sub(out=qrope[0:32, h, :], in0=t1[:], in1=t2[:])
                        nc.vector.tensor_mul(out=t1[:], in0=qr_ps[0:32], in1=sin_sb[:])
                        nc.vector.tensor_mul(out=t2[:], in0=od[:], in1=cos_sb[:])
                        nc.vector.tensor_add(out=qrope[32:64, h, :], in0=t1[:], in1=t2[:])

        g_a.close()   # free xTb, lat weights, qup, wkTT, xb chunks

        # masks + wv (B-scope) and phase-C/D weight prefetch
        g_m = ES()
        p_m = g_m.enter_context(tc.tile_pool(name="pm", bufs=1, side="right"))
        m1_sb = p_m.tile([P, 8, NLOC], BF16, tag="m1")
        nc.sync.dma_start(out=m1_sb[:], in_=mask1.rearrange("a p n -> p a n"))
        m2_sb = p_m.tile([P, 8, 256], BF16, tag="m2")
        nc.sync.dma_start(out=m2_sb[:], in_=mask2.rearrange("a p n -> p a n"))
        wv_sb = p_m.tile([P, 2, H * VD], F32R, tag="wv")
        nc.sync.dma_start(out=wv_sb[:], in_=wv_w[:])
        kvtm = p_m.tile([P, 16, KVLR], F32R, tag="kvtm")
        with tc.tile_pool(name="pstp", bufs=2, space="PSUM") as ps_tp:
            for kt in range(16):
                for dsi in range(2):
                    tp = ps_tp.tile([P, P], F32R, tag="tp")
                    nc.tensor.transpose(tp[:], kvn[:, dsi, kt * 128:(kt + 1) * 128],
                                        identr[:])
                    nc.vector.tensor_copy(
                        out=kvtm[:, kt, dsi * 128:(dsi + 1) * 128], in_=tp[:])

        p_wC1 = ES()
        p_cproj = p_wC1.enter_context(tc.tile_pool(name="pcproj", bufs=1))
        cw_sb = p_cproj.tile([P, 8, D], F32R, tag="cproj")
        nc.gpsimd.dma_start(out=cw_sb[:], in_=cproj_w[:])
        gw_sb = p_cproj.tile([P, 8, E], F32R, tag="gw")
        nc.gpsimd.dma_start(out=gw_sb[:], in_=gate_w[:])

        nc.scalar.dma_start(out=dbg_kvn[:], in_=kvn[:])
        nc.scalar.dma_start(out=dbg_krx[:], in_=krx[:])
        nc.scalar.dma_start(out=dbg_qabs[:], in_=qabs[:])
        # ================= phase B: attention =================
        with tc.tile_pool(name="psst", bufs=2, space="PSUM") as ps_st, \
             tc.tile_pool(name="psol", bufs=1, space="PSUM") as ps_ol, \
             tc.tile_pool(name="psden", bufs=1, space="PSUM") as ps_den, \
             tc.tile_pool(name="patt", bufs=2) as p_att:
            for h in range(H):
                olA0 = ps_ol.tile([P, 256], F32, tag="olA0")
                olA1 = ps_ol.tile([P, 256], F32, tag="olA1")
                olB0 = ps_ol.tile([P, 256], F32, tag="olB0")
                olB1 = ps_ol.tile([P, 256], F32, tag="olB1")
                ols = [olA0, olA1, olB0, olB1]
                denA = ps_den.tile([1, 256], F32, tag="denA")
                denB = ps_den.tile([1, 256], F32, tag="denB")
                for kt in range(16):
                    slab1 = kt < 8
                    w = NLOC if slab1 else 256
                    qof = 0 if slab1 else 256
                    kc = slice(kt * 128, (kt + 1) * 128)
                    st = ps_st.tile([P, NLOC], F32, tag="st")
                    nc.tensor.matmul(st[:, :w], kvn[:, 0, kc],
                                     qabs[:, 0 + 2 * h, qof:NLOC], start=True, stop=False)
                    nc.tensor.matmul(st[:, :w], kvn[:, 1, kc],
                                     qabs[:, 1 + 2 * h, qof:NLOC], start=False, stop=False)
                    nc.tensor.matmul(st[:, :w], krx[:, kc],
                                     qrope[:, h, qof:NLOC], start=False, stop=True)
                    msb = m1_sb[:, kt, :] if slab1 else m2_sb[:, kt - 8, :]
                    nc.vector.tensor_tensor(out=st[:, :w], in0=st[:, :w], in1=msb,
                                            op=AL.add)
                    pt = p_att.tile([P, NLOC], F32R, tag="pt")
                    nc.scalar.activation(out=pt[:, :w], in_=st[:, :w], func=AF.Exp)
                    if slab1:
                        nc.tensor.matmul(denA[:], onesr_sb[:], pt[:, 0:256],
                                         start=(kt == 0), stop=(kt == 7))
                        for half in range(2):
                            nc.tensor.matmul(ols[half][:],
                                             kvtm[:, kt, half * 128:(half + 1) * 128],
                                             pt[:, 0:256], start=(kt == 0), stop=(kt == 7))
                        pB = pt[:, 256:NLOC]
                    else:
                        pB = pt[:, 0:256]
                    nc.tensor.matmul(denB[:], onesr_sb[:], pB,
                                     start=(kt == 0), stop=(kt == 15))
                    for half in range(2):
                        nc.tensor.matmul(ols[2 + half][:],
                                         kvtm[:, kt, half * 128:(half + 1) * 128],
                                         pB, start=(kt == 0), stop=(kt == 15))
                row = p_sc.tile([1, NLOC], F32, tag="row")
                nc.vector.reciprocal(out=row[:, 0:256], in_=denA[:])
                nc.vector.reciprocal(out=row[:, 256:NLOC], in_=denB[:])
                for s in range(2):
                    rb = brow(ps_st, p_att, row[:, s * 256:(s + 1) * 256],
                              256, tag="rb", ptag="st")
                    olsb = p_att.tile([P, 2, 256], F32R, tag="olsb")
                    for half in range(2):
                        nc.vector.tensor_tensor(out=olsb[:, half, :],
                                                in0=ols[2 * s + half][:],
                                                in1=rb[:], op=AL.mult)
                    yp = ps_st.tile([P, NLOC], F32, tag="st")
                    nc.tensor.matmul(yp[:, 0:256], wv_sb[:, 0, h * VD:(h + 1) * VD],
                                     olsb[:, 0, :], start=True, stop=False)
                    nc.tensor.matmul(yp[:, 0:256], wv_sb[:, 1, h * VD:(h + 1) * VD],
                                     olsb[:, 1, :], start=False, stop=True)
                    nc.vector.tensor_copy(out=yT[:, h, s * 256:(s + 1) * 256],
                                          in_=yp[:, 0:256])

        nc.sync.dma_start(out=dbg_yT[:], in_=yT[:])
        g_m.close()
        g_q.close()
        g_kv.close()

        # ================= phase C: cproj + rms2 + gate =================
        p_wC2 = ES()
        p_sh = p_wC2.enter_context(tc.tile_pool(name="psh", bufs=1))
        sel8_sb = p_sh.tile([E, E * P], BF16, tag="sel8")
        nc.gpsimd.dma_start(out=sel8_sb[:], in_=sel8[:])
        sh1_sb = p_sh.tile([P, 8, INTER * NSH], FP8, tag="sh1")
        nc.gpsimd.dma_start(out=sh1_sb[:], in_=shw1[:])
        sh3_sb = p_sh.tile([P, 8, INTER * NSH], FP8, tag="sh3")
        nc.gpsimd.dma_start(out=sh3_sb[:], in_=shw3[:])
        sh2_sb = p_sh.tile([P, 8, D], BF16, tag="sh2")
        nc.gpsimd.dma_start(out=sh2_sb[:], in_=shw2[:])
        with tc.tile_pool(name="pscC", bufs=2) as p_scC, \
             tc.tile_pool(name="psmm", bufs=3, space="PSUM") as ps_mm, \
             tc.tile_pool(name="psrow2", bufs=1, space="PSUM") as ps_row2, \
             tc.tile_pool(name="psbcm", bufs=1, space="PSUM") as ps_bcm, \
             tc.tile_pool(name="pstp2", bufs=2, space="PSUM") as ps_tp2:

            nc.gpsimd.dma_start(out=x2[:], in_=xT.rearrange("(a p) n -> p a n", p=P))
            for ft in range(8):
                op = ps_mm.tile([P, NLOC], F32, tag="mm")
                for ds in range(8):
                    nc.tensor.matmul(op[:], cw_sb[:, ds, ft * 128:(ft + 1) * 128],
                                     yT[:, ds, :], start=(ds == 0), stop=(ds == 7))
                nc.vector.tensor_add(out=x2[:, ft, :], in0=op[:], in1=x2[:, ft, :])

            # rms2 (exact fp32 squares: feeds the gate)
            ss2 = ps_row2.tile([1, NLOC], F32, tag="row2")
            for ds in range(8):
                xsq = p_scC.tile([P, NLOC], F32R, tag="xsq32")
                nc.vector.tensor_mul(out=xsq[:], in0=x2[:, ds, :], in1=x2[:, ds, :])
                nc.tensor.matmul(ss2[:], onesr_sb[:], xsq[:],
                                 start=(ds == 0), stop=(ds == 7))
            row = p_sc.tile([1, NLOC], F32, tag="row")
            nc.scalar.activation(out=row[:], in_=ss2[:],
                                 func=AF.Sqrt, bias=eps1[:], scale=1.0 / D)
            nc.vector.reciprocal(out=srow2[:], in_=row[:])
            row2 = p_sc.tile([1, NLOC], F32, tag="row")
            nc.vector.tensor_scalar_mul(out=row2[:], in0=srow2[:], scalar1=float(HS))
            s2b = brow(ps_bcm, p_sc, row2[:], NLOC, tag="bcsb")
            for ds in range(8):
                nc.vector.tensor_tensor(out=h2f8[:, ds, :], in0=x2[:, ds, :],
                                        in1=s2b[:], op=AL.mult)

            # gate
            gp = ps_mm.tile([P, NLOC], F32, tag="mm")
            for ds in range(8):
                nc.tensor.matmul(gp[:E], gw_sb[:, ds, :], x2[:, ds, :],
                                 start=(ds == 0), stop=(ds == 7))
            g_sb = p_scC.tile([E, NLOC], F32, tag="gsb")
            nc.vector.tensor_copy(out=g_sb[:], in_=gp[:E])
            for q4 in range(4):
                tp = ps_tp2.tile([P, P], F32, tag="tp2")
                nc.tensor.transpose(tp[:, 0:E], g_sb[:, q4 * 128:(q4 + 1) * 128],
                                    identf[0:E, 0:E])
                gt = p_scC.tile([P, E], F32, tag="gt")
                nc.vector.tensor_copy(out=gt[:], in_=tp[:, 0:E])
                s2tp = ps_tp2.tile([P, P], F32, tag="tp2")
                nc.tensor.transpose(s2tp[:, 0:1], srow2[:, q4 * 128:(q4 + 1) * 128],
                                    identf[0:1, 0:1])
                s2c = p_scC.tile([P, 1], F32, tag="s2c")
                nc.vector.tensor_copy(out=s2c[:], in_=s2tp[:, 0:1])
                nc.vector.tensor_scalar_mul(out=gt[:], in0=gt[:], scalar1=s2c[:])
                mx = p_scC.tile([P, 4], F32, tag="mx")
                nc.vector.tensor_reduce(out=mx[:, 0:1], in_=gt[:],
                                        axis=mybir.AxisListType.X, op=AL.max)
                nc.vector.tensor_scalar_mul(out=mx[:, 1:2], in0=mx[:, 0:1], scalar1=-1.0)
                e8 = p_scC.tile([P, E], F32, tag="e8")
                nc.scalar.activation(out=e8[:], in_=gt[:], func=AF.Exp,
                                     bias=mx[:, 1:2], accum_out=mx[:, 2:3])
                nc.vector.reciprocal(out=mx[:, 3:4], in_=mx[:, 2:3])
                srt = p_scC.tile([P, E], F32, tag="srt")
                nc.vector.max(out=srt[:], in_=e8[:])
                cwq = p_scC.tile([P, E], F32, tag="cwq")
                nc.vector.tensor_scalar(out=cwq[:], in0=e8[:], scalar1=srt[:, 1:2],
                                        scalar2=None, op0=AL.is_ge)
                nc.vector.tensor_mul(out=cwq[:], in0=cwq[:], in1=e8[:])
                nc.vector.tensor_scalar_mul(out=cwq[:], in0=cwq[:], scalar1=mx[:, 3:4])
                tp2 = ps_tp2.tile([P, P], F32, tag="tp2")
                nc.tensor.transpose(tp2[0:E, 0:P], cwq[:], identf[:])
                nc.vector.tensor_copy(out=cwT[:, q4 * 128:(q4 + 1) * 128],
                                      in_=tp2[0:E, 0:P])

        nc.sync.dma_start(out=dbg_x2[:], in_=x2[:])
        g_yT.close()

        # ================= phase D: MoE (fp8 DoubleRow, 2-pass) =================
        with tc.tile_pool(name="pscD", bufs=2) as p_scD, \
             tc.tile_pool(name="pwe2", bufs=2) as p_we2, \
             tc.tile_pool(name="phsh", bufs=1) as p_hsh:

            hshs = []
            cwb_sh = p_scD.tile([P, NLOC], F32, tag="cwbsh")
            nc.vector.memset(cwb_sh[:], float(1.0 / QW))

            def mlp13(w1s, w3s, hsh8, nft, cwb):
                for ft in range(nft):
                    g1 = ps_g.tile([P, NLOC], F32, tag="g1")
                    g3 = ps_g.tile([P, NLOC], F32, tag="g3")
                    for dp in range(4):
                        nc.tensor.matmul(g1[:], w1s[:, 2 * dp:2 * dp + 2,
                                                    ft * 128:(ft + 1) * 128],
                                         h2f8[:, 2 * dp:2 * dp + 2, :],
                                         perf_mode=DR, start=(dp == 0), stop=(dp == 3))
                    for dp in range(4):
                        nc.tensor.matmul(g3[:], w3s[:, 2 * dp:2 * dp + 2,
                                                    ft * 128:(ft + 1) * 128],
                                         h2f8[:, 2 * dp:2 * dp + 2, :],
                                         perf_mode=DR, start=(dp == 0), stop=(dp == 3))
                    sl = p_scD.tile([P, NLOC], F32, tag="silu")
                    nc.scalar.activation(out=sl[:], in_=g1[:], func=AF.Silu,
                                         scale=float(1.0 / (HS * QW)))
                    tmp = p_scD.tile([P, NLOC], F32, tag="tmp")
                    nc.vector.tensor_mul(out=tmp[:], in0=sl[:], in1=g3[:])
                    nc.vector.tensor_tensor(out=hsh8[:, ft, :], in0=tmp[:],
                                            in1=cwb[:], op=AL.mult)

            # pass 1: routed experts
            p1 = ES()
            p_we = p1.enter_context(tc.tile_pool(name="pwe", bufs=2))
            ps_g = p1.enter_context(tc.tile_pool(name="psg", bufs=2, space="PSUM"))
            ps_bcd = p1.enter_context(tc.tile_pool(name="psbcd", bufs=2, space="PSUM"))
            for e in range(E):
                e1_sb = p_we.tile([P, 8, INTER], FP8, tag="we1")
                nc.sync.dma_start(out=e1_sb[:], in_=ew1[e])
                e3_sb = p_we.tile([P, 8, INTER], FP8, tag="we3")
                nc.sync.dma_start(out=e3_sb[:], in_=ew3[e])
                hsh8 = p_hsh.tile([P, 4, NLOC], BF16, tag=f"hsh{e}")
                hshs.append(hsh8)
                cwp = ps_bcd.tile([P, NLOC], F32, tag="bc")
                nc.tensor.matmul(cwp[:], sel8_sb[:, e * P:(e + 1) * P], cwT[:],
                                 start=True, stop=True)
                cwb = p_scD.tile([P, NLOC], F32, tag="cwb")
                nc.vector.tensor_copy(out=cwb[:], in_=cwp[:])
                mlp13(e1_sb, e3_sb, hsh8, 4, cwb)
            # pass 1b: shared expert (cw = 1)
            hsh_sh = p_hsh.tile([P, 8, NLOC], BF16, tag="hshsh")
            mlp13(sh1_sb, sh3_sb, hsh_sh, 8, cwb_sh)

            p1.close()
            # pass 2: w2 accumulation across experts + shared, then residual
            with tc.tile_pool(name="psacc", bufs=1, space="PSUM") as ps_acc:
                acc = ps_acc.tile([P, 8, NLOC], F32, tag="acc")
                for e in range(E):
                    e2_sb = p_we2.tile([P, 4, D], BF16, tag="we2")
                    nc.sync.dma_start(out=e2_sb[:], in_=ew2[e])
                    for ft in range(8):
                        for ds in range(4):
                            nc.tensor.matmul(acc[:, ft, :],
                                             e2_sb[:, ds, ft * 128:(ft + 1) * 128],
                                             hshs[e][:, ds, :],
                                             start=(e == 0 and ds == 0), stop=False)
                for ft in range(8):
                    for ds in range(8):
                        nc.tensor.matmul(acc[:, ft, :],
                                         sh2_sb[:, ds, ft * 128:(ft + 1) * 128],
                                         hsh_sh[:, ds, :],
                                         start=False, stop=(ds == 7))
                for ft in range(8):
                    msb = p_scD.tile([P, NLOC], F32, tag="msb")
                    nc.scalar.activation(out=msb[:], in_=acc[:, ft, :], func=AF.Copy,
                                         scale=float(1.0 / (HS * QW)))
                    nc.vector.tensor_add(out=x2[:, ft, :], in0=msb[:], in1=x2[:, ft, :])

            nc.sync.dma_start(out=out_xT.rearrange("(a p) n -> p a n", p=P), in_=x2[:])

        p_wC2.close()
        p_wC1.close()
        g_x.close()

    nc.finalize()
    return nc


# ============================ host side ============================

_CACHE = {}


def _prep_shared(inputs):
    perm = _rope_perm()
    f = np.float32
    latent_w = (np.asarray(inputs["latent_w"], f)
                * np.asarray(inputs["rmsn1_w"], f)[:, None]).copy()
    latent_w[:, QLR + KVLR:] = latent_w[:, QLR + KVLR:][:, perm]
    q_up = (np.asarray(inputs["q_up_w"], f)
            * np.asarray(inputs["q_norm_w"], f)[:, None]).copy()
    for h in range(H):
        c0 = h * QKD + NOPE
        q_up[:, c0:c0 + ROPE] = q_up[:, c0:c0 + ROPE][:, perm]
    kv_up = (np.asarray(inputs["kv_up_w"], f)
             * np.asarray(inputs["kv_norm_w"], f)[:, None])
    wk = np.stack([kv_up[:, h * (NOPE + VD):h * (NOPE + VD) + NOPE].T
                   for h in range(H)], axis=1)   # [NOPE, H, KVLR]
    wv = np.concatenate([kv_up[:, h * (NOPE + VD) + NOPE:(h + 1) * (NOPE + VD)]
                         for h in range(H)], axis=1)
    r2 = np.asarray(inputs["rmsn2_w"], f)[:, None]
    latw_t = _tile_w(latent_w)           # [P, 8, 704]
    shared = {
        "ident": np.eye(P, dtype=f),
        "identb": np.eye(P, dtype=f).astype(BFNP),
        "onesbf": np.ones((P, 1), dtype=BFNP),
        "latq_w": np.ascontiguousarray(latw_t[:, :, :QLR]).astype(BFNP),
        "latkv_w": np.ascontiguousarray(latw_t[:, :, QLR:]).astype(BFNP),
        "q_up": _tile_w(q_up),
        "wkTT": np.ascontiguousarray(wk.astype(f)),
        "wv_w": _tile_w(wv.astype(f)),
        "cproj_w": _tile_w(np.asarray(inputs["c_proj_w"], f)),
        "gate_w": _tile_w(np.asarray(inputs["gate_w"], f) * r2),
        "shw1": _f8(_tile_w(np.asarray(inputs["sh_w1"], f) * r2) * QW),
        "shw3": _f8(_tile_w(np.asarray(inputs["sh_w3"], f) * r2) * QW),
        "shw2": (_tile_w(np.asarray(inputs["sh_w2"], f)) * QW).astype(BFNP),
        "ew1": np.stack([_f8(_tile_w(np.asarray(inputs["e_w1"], f)[e] * r2) * QW)
                         for e in range(E)]),
        "ew3": np.stack([_f8(_tile_w(np.asarray(inputs["e_w3"], f)[e] * r2) * QW)
                         for e in range(E)]),
        "ew2": np.stack([(_tile_w(np.asarray(inputs["e_w2"], f)[e]) * QW).astype(BFNP)
                         for e in range(E)]),
        "sel8": np.repeat(np.eye(E, dtype=f) * (1.0 / QW), P, axis=1)
                  .reshape(E, E * P).astype(BFNP),
        "ones1f": np.ones((1, P), dtype=f),
        "onesr": np.ones((P, 1), dtype=f),
    }
    return shared


def _prep_core(inputs, c):
    f = np.float32
    pos = _core_positions(c)
    b = c // 4
    gidx = b * T + pos
    xflat = np.asarray(inputs["x"], dtype=f).reshape(N, D)
    xT_c = np.ascontiguousarray(xflat[gidx].T)
    xTb_c = np.ascontiguousarray(
        xflat[gidx].reshape(NLOC, 8, P).transpose(2, 1, 0)).astype(BFNP)
    xb_c = np.ascontiguousarray(
        xflat[b * T:(b + 1) * T].reshape(T, 8, P).transpose(2, 1, 0)
    ).astype(BFNP)
    cosb = np.ascontiguousarray(np.asarray(inputs["freqs_cos"], f).T).astype(BFNP)
    sinb = np.ascontiguousarray(np.asarray(inputs["freqs_sin"], f).T).astype(BFNP)
    cosT = np.ascontiguousarray(np.asarray(inputs["freqs_cos"], f)[pos].T).astype(BFNP)
    sinT = np.ascontiguousarray(np.asarray(inputs["freqs_sin"], f)[pos].T).astype(BFNP)
    k_abs = (np.arange(8)[:, None] * 128 + np.arange(P)[None, :])
    m1 = np.where(k_abs[:, :, None] <= pos[None, None, :], 0.0, NEG)
    k_abs2 = ((np.arange(8, 16))[:, None] * 128 + np.arange(P)[None, :])
    m2 = np.where(k_abs2[:, :, None] <= pos[None, None, 256:], 0.0, NEG)
    return {
        "xT": xT_c, "xTb": xTb_c, "xb": xb_c, "cosbT": cosb, "sinbT": sinb,
        "cosT": cosT, "sinT": sinT,
        "mask1": m1.astype(BFNP),
        "mask2": m2.astype(BFNP),
    }, gidx


def run(inputs, trace=False, **kw):
    if "nc" not in _CACHE:
        _CACHE["nc"] = build()
    nc = _CACHE["nc"]
    shared = _prep_shared(inputs)
    in_maps = []
    gidxs = []
    for c in range(NCORES):
        m, gidx = _prep_core(inputs, c)
        m.update(shared)
        in_maps.append(m)
        gidxs.append(gidx)
    res = run_bass_kernel_spmd(nc, in_maps, core_ids=list(range(NCORES)),
                               trace=trace, **kw)
    full = np.empty((N, D), dtype=np.float32)
    for c in range(NCORES):
        full[gidxs[c]] = np.asarray(res.results[c]["out_xT"], np.float32).T
    return full.reshape(B, T, D), res


def kernel(**inputs):
    out, _ = run(inputs)
    return out



# revision 26
# speedup vs baseline: 1.1845x; 1.1845x over previous
"""MLA + DeepSeekMoE block on 8 trn2 NeuronCores (Bass/Tile SPMD).

Token-sharded across 8 cores (512 tokens each, causally balanced stripes).
v2: no collective -- each core recomputes the KV latent (kv256+rope64) for
all 2048 batch tokens from a bf16 copy of x (MLA absorption trick).
Attention operands in bf16 (same PE rate as fp32r, half the SBUF/DVE).
MoE (shared + all 8 routed experts, computed densely) in fp8-e4m3 with
DoubleRow matmuls; expert w2 outputs accumulate across experts in PSUM.
cproj/gate stay fp32r so the gate top-2 selection matches the reference.

v3 perf changes (numerics-preserving):
  - removed all debug outputs (10+ MB of HBM writes at phase boundaries)
  - all row stats (rms / softmax denom) computed as [128, N] via all-ones
    [128,128] stationary matmuls: kills 1-partition DVE ops (3.3us each),
    the brow broadcast matmul+copy, and the slow M=1 matmuls
  - causal mask applied multiplicatively after exp (shorter PSUM chain)
  - rope rotation batched across heads (A3) / chunks (A2) on full tiles
  - squares for rms stats moved to the idle Scalar engine (Square act)
  - softmax scale folded into q_up on the host; attention output
    normalized after the wv matmul (one less DVE op per head-slab)
"""

import numpy as np
import ml_dtypes

import concourse.bacc as bacc
import concourse.mybir as mybir
from concourse.tile import TileContext
from concourse.bass_utils import run_bass_kernel_spmd

# ---- problem constants ----
D = 1024; H = 8; QLR = 384; KVLR = 256; NOPE = 128; ROPE = 64; VD = 128
E = 8; TOPK = 2; INTER = 512; NSH = 2; B = 2; T = 2048; QKD = NOPE + ROPE
N = B * T
NCORES = 8
NLOC = N // NCORES          # 512 tokens per core
P = 128
EPS = 1e-6
SCALE = 1.0 / np.sqrt(QKD)
QW = 64.0                   # fp8 weight scale
HS = 16.0                   # fp8 h2 / hsh scale

F32 = mybir.dt.float32
F32R = mybir.dt.float32r
BF16 = mybir.dt.bfloat16
FP8 = mybir.dt.float8e4
DR = mybir.MatmulPerfMode.DoubleRow
E4NP = ml_dtypes.float8_e4m3
BFNP = ml_dtypes.bfloat16

# MoE fp8 scale plan:
#   h2f8 = HS*h2;  w1,w3,w2 fp8 x QW
#   g1 psum = HS*QW*g1_true -> silu(scale=1/(HS*QW)) = silu_true
#   tmp = silu*g3 = HS*QW*hsh_true
#   hsh8 = tmp * cwb,  cwb = cw/QW  (sel8 prescaled 1/QW; shared cw=1)
#   acc psum = hsh8 @ w2(xQW) = HS*QW*(cw-weighted moe) -> copy scale 1/(HS*QW)


def _rope_perm():
    return np.concatenate([np.arange(0, ROPE, 2), np.arange(1, ROPE, 2)])


def _core_positions(c):
    j = c % 4
    return np.concatenate([np.arange(j * 256, (j + 1) * 256),
                           np.arange((7 - j) * 256, (8 - j) * 256)])


def _tile_w(w):
    """[K, F] row-major -> [128, K//128, F] partition-major contiguous."""
    K, F = w.shape
    return np.ascontiguousarray(w.reshape(K // P, P, F).transpose(1, 0, 2))


def _f8(a):
    return np.ascontiguousarray(np.asarray(a, np.float32)).astype(E4NP)


# ============================ device program ============================

def build():
    from contextlib import ExitStack as ES
    nc = bacc.Bacc(name="mla_moe_v3")

    # ---- I/O ----
    xT = nc.dram_tensor("xT", [D, NLOC], F32R, kind="ExternalInput")
    xTb = nc.dram_tensor("xTb", [P, 8, NLOC], BF16, kind="ExternalInput")
    xb = nc.dram_tensor("xb", [P, 8, T], BF16, kind="ExternalInput")
    cosbT = nc.dram_tensor("cosbT", [ROPE // 2, 4, 512], BF16, kind="ExternalInput")
    sinbT = nc.dram_tensor("sinbT", [ROPE // 2, 4, 512], BF16, kind="ExternalInput")
    cosT4 = nc.dram_tensor("cosT4", [ROPE // 2, 4, NLOC], BF16, kind="ExternalInput")
    sinT4 = nc.dram_tensor("sinT4", [ROPE // 2, 4, NLOC], BF16, kind="ExternalInput")
    mask1 = nc.dram_tensor("mask1", [8, P, NLOC], BF16, kind="ExternalInput")
    mask2 = nc.dram_tensor("mask2", [8, P, 256], BF16, kind="ExternalInput")
    ident = nc.dram_tensor("ident", [P, P], F32, kind="ExternalInput")
    latq_w = nc.dram_tensor("latq_w", [P, 8, QLR], BF16, kind="ExternalInput")
    latkv_w = nc.dram_tensor("latkv_w", [P, 8, KVLR + ROPE], BF16, kind="ExternalInput")
    q_up = nc.dram_tensor("q_up", [P, 3, H * QKD], F32R, kind="ExternalInput")
    wkTT = nc.dram_tensor("wkTT", [P, H, KVLR], F32R, kind="ExternalInput")
    wv_w = nc.dram_tensor("wv_w", [P, 2, H * VD], F32R, kind="ExternalInput")
    cproj_w = nc.dram_tensor("cproj_w", [P, 8, D], F32R, kind="ExternalInput")
    gate_w = nc.dram_tensor("gate_w", [P, 8, E], F32R, kind="ExternalInput")
    shw1 = nc.dram_tensor("shw1", [P, 8, INTER * NSH], FP8, kind="ExternalInput")
    shw3 = nc.dram_tensor("shw3", [P, 8, INTER * NSH], FP8, kind="ExternalInput")
    shw2 = nc.dram_tensor("shw2", [P, 8, D], BF16, kind="ExternalInput")
    ew1 = nc.dram_tensor("ew1", [E, P, 8, INTER], FP8, kind="ExternalInput")
    ew3 = nc.dram_tensor("ew3", [E, P, 8, INTER], FP8, kind="ExternalInput")
    ew2 = nc.dram_tensor("ew2", [E, P, 4, D], BF16, kind="ExternalInput")
    sel8 = nc.dram_tensor("sel8", [E, E * P], BF16, kind="ExternalInput")
    out_xT = nc.dram_tensor("out_xT", [D, NLOC], F32R, kind="ExternalOutput")

    AL = mybir.AluOpType
    AF = mybir.ActivationFunctionType

    with TileContext(nc) as tc, \
         nc.allow_low_precision(reason="f32r rows / bf16+fp8 activations by design"), \
         tc.tile_pool(name="const", bufs=1) as p_const, \
         tc.tile_pool(name="psc", bufs=2) as p_sc:

        # right-side LIFO stack (open order = reverse close order)
        g_x = ES();  p_x = g_x.enter_context(tc.tile_pool(name="px", bufs=1, side="right"))
        g_yT = ES(); p_yt = g_yT.enter_context(tc.tile_pool(name="pyt", bufs=1, side="right"))
        g_kv = ES(); p_kv = g_kv.enter_context(tc.tile_pool(name="pkv", bufs=1, side="right"))
        g_q = ES();  p_q = g_q.enter_context(tc.tile_pool(name="pq", bufs=1, side="right"))
        g_a = ES()
        p_wA = g_a.enter_context(tc.tile_pool(name="pwA", bufs=1, side="right"))
        p_qu = g_a.enter_context(tc.tile_pool(name="pqu", bufs=1, side="right"))
        p_xb = g_a.enter_context(tc.tile_pool(name="pxb", bufs=2, side="right"))

        # ---- constants ----
        identf = p_const.tile([P, P], F32, tag="identf")
        nc.gpsimd.dma_start(out=identf[:], in_=ident[:])
        identr = p_const.tile([P, P], F32R, tag="identr")
        nc.vector.tensor_copy(out=identr[:], in_=identf[:])
        ones128f = p_const.tile([P, P], F32, tag="ones128f")
        nc.vector.memset(ones128f[:], 1.0)
        ones128b = p_const.tile([P, P], BF16, tag="ones128b")
        nc.vector.tensor_copy(out=ones128b[:], in_=ones128f[:])
        ones128r = p_const.tile([P, P], F32R, tag="ones128r")
        nc.vector.tensor_copy(out=ones128r[:], in_=ones128f[:])
        eps128 = p_const.tile([P, 1], F32, tag="eps128")
        nc.vector.memset(eps128[:], EPS)


        # ---- persistent activations ----
        x2 = p_x.tile([P, 8, NLOC], F32R, tag="x2")
        h2f8 = p_x.tile([P, 8, NLOC], FP8, tag="h2f8")
        cwT = p_x.tile([E, NLOC], BF16, tag="cwT")
        yT = p_yt.tile([P, H, NLOC], F32R, tag="yT")

        kvn = p_kv.tile([P, 2, T], F32R, tag="kvn")
        krx = p_kv.tile([ROPE, 4, 512], BF16, tag="krx")

        qabs = p_q.tile([P, 2 * H, NLOC], F32R, tag="qabs")
        qrope = p_q.tile([ROPE, H, NLOC], BF16, tag="qrope")

        cos4_sb = p_wA.tile([ROPE // 2, 4, NLOC], BF16, tag="cos4")
        nc.gpsimd.dma_start(out=cos4_sb[:], in_=cosT4[:])
        sin4_sb = p_wA.tile([ROPE // 2, 4, NLOC], BF16, tag="sin4")
        nc.gpsimd.dma_start(out=sin4_sb[:], in_=sinT4[:])
        cosb_sb = p_wA.tile([ROPE // 2, 4, 512], BF16, tag="cosb")
        nc.gpsimd.dma_start(out=cosb_sb[:], in_=cosbT[:])
        sinb_sb = p_wA.tile([ROPE // 2, 4, 512], BF16, tag="sinb")
        nc.gpsimd.dma_start(out=sinb_sb[:], in_=sinbT[:])

        xTb_sb = p_wA.tile([P, 8, NLOC], BF16, tag="xTb")
        nc.sync.dma_start(out=xTb_sb[:], in_=xTb[:])
        latq_sb = p_wA.tile([P, 8, QLR], BF16, tag="latqw")
        nc.sync.dma_start(out=latq_sb[:], in_=latq_w[:])
        latkv_sb = p_wA.tile([P, 8, KVLR + ROPE], BF16, tag="latkvw")
        nc.sync.dma_start(out=latkv_sb[:], in_=latkv_w[:])
        wkTT_sb = p_wA.tile([P, H, KVLR], F32R, tag="wkTT")
        nc.sync.dma_start(out=wkTT_sb[:], in_=wkTT[:])

        def rstat(row_ps, scale):
            """[128,N] PSUM sum-of-squares -> [128,N] SBUF rsqrt(mean+eps)."""
            w = row_ps.shape[-1]
            sb = p_sc.tile([P, w], F32, tag="rstat")
            nc.scalar.activation(out=sb[:], in_=row_ps[:],
                                 func=mybir.ActivationFunctionType.Sqrt,
                                 bias=eps128[:], scale=scale)
            nc.vector.reciprocal(out=sb[:], in_=sb[:])
            return sb

        # ================= phase A =================
        with tc.tile_pool(name="actA", bufs=1) as p_actA, \
             tc.tile_pool(name="pr", bufs=1) as p_r, \
             tc.tile_pool(name="pslat", bufs=2, space="PSUM") as ps_lat, \
             tc.tile_pool(name="psrow", bufs=2, space="PSUM") as ps_row:

            # ---- A1: local rms1 + q latents (bf16 inputs; scores-only) ----
            ss_ps = ps_row.tile([P, NLOC], F32, tag="ss")
            for ds in range(8):
                xsq = p_sc.tile([P, NLOC], BF16, tag="xsq")
                nc.scalar.activation(out=xsq[:], in_=xTb_sb[:, ds, :], func=AF.Square)
                nc.tensor.matmul(ss_ps[:], ones128b[:], xsq[:],
                                 start=(ds == 0), stop=(ds == 7))
            s1loc = rstat(ss_ps, 1.0 / D)

            qln = p_actA.tile([P, 3, NLOC], F32R, tag="qln")
            for ft in range(3):
                lp = ps_lat.tile([P, NLOC], F32, tag="lat")
                for ds in range(8):
                    nc.tensor.matmul(lp[:], latq_sb[:, ds, ft * 128:(ft + 1) * 128],
                                     xTb_sb[:, ds, :], start=(ds == 0), stop=(ds == 7))
                nc.vector.tensor_tensor(out=qln[:, ft, :], in0=lp[:],
                                        in1=s1loc[:], op=AL.mult)
            sq_ps = ps_row.tile([P, NLOC], F32, tag="ss")
            for t in range(3):
                xsq = p_sc.tile([P, NLOC], BF16, tag="xsq")
                nc.scalar.activation(out=xsq[:], in_=qln[:, t, :], func=AF.Square)
                nc.tensor.matmul(sq_ps[:], ones128b[:], xsq[:],
                                 start=(t == 0), stop=(t == 2))
            sqb = rstat(sq_ps, 1.0 / QLR)
            for t in range(3):
                nc.vector.tensor_tensor(out=qln[:, t, :], in0=qln[:, t, :],
                                        in1=sqb[:], op=AL.mult)

            # ---- A2: batch kv latents, 4 chunks of 512 tokens ----
            kre = p_actA.tile([ROPE // 2, 4, 512], BF16, tag="re")
            kro = p_actA.tile([ROPE // 2, 4, 512], BF16, tag="ro")
            t1r = p_actA.tile([ROPE // 2, 4, 512], BF16, tag="t1r")
            t2r = p_actA.tile([ROPE // 2, 4, 512], BF16, tag="t2r")
            for c4 in range(4):
                cs = slice(c4 * 512, (c4 + 1) * 512)
                xbc = p_xb.tile([P, 8, NLOC], BF16, tag="xbc")
                nc.sync.dma_start(out=xbc[:], in_=xb[:, :, cs])
                ssb = ps_row.tile([P, NLOC], F32, tag="ss")
                for ds in range(8):
                    xsq = p_sc.tile([P, NLOC], BF16, tag="xsq")
                    nc.scalar.activation(out=xsq[:], in_=xbc[:, ds, :], func=AF.Square)
                    nc.tensor.matmul(ssb[:], ones128b[:], xsq[:],
                                     start=(ds == 0), stop=(ds == 7))
                s1b = rstat(ssb, 1.0 / D)

                kvt = p_r.tile([P, 2, NLOC], F32, tag="kvt")
                for i in range(2):
                    lp = ps_lat.tile([P, NLOC], F32, tag="lat")
                    for ds in range(8):
                        nc.tensor.matmul(lp[:], latkv_sb[:, ds, i * 128:(i + 1) * 128],
                                         xbc[:, ds, :], start=(ds == 0), stop=(ds == 7))
                    nc.vector.tensor_tensor(out=kvt[:, i, :], in0=lp[:],
                                            in1=s1b[:], op=AL.mult)
                kss = ps_row.tile([P, NLOC], F32, tag="ss")
                for i in range(2):
                    xsq = p_sc.tile([P, NLOC], BF16, tag="xsq")
                    nc.scalar.activation(out=xsq[:], in_=kvt[:, i, :], func=AF.Square)
                    nc.tensor.matmul(kss[:], ones128b[:], xsq[:],
                                     start=(i == 0), stop=(i == 1))
                skvb = rstat(kss, 1.0 / KVLR)
                for i in range(2):
                    nc.vector.tensor_tensor(out=kvn[:, i, cs], in0=kvt[:, i, :],
                                            in1=skvb[:], op=AL.mult)
                # rope rows x s1 -> staged; rotation batched after the loop
                lp = ps_lat.tile([P, NLOC], F32, tag="lat")
                for ds in range(8):
                    nc.tensor.matmul(lp[:ROPE], latkv_sb[:, ds, KVLR:KVLR + ROPE],
                                     xbc[:, ds, :], start=(ds == 0), stop=(ds == 7))
                nc.vector.tensor_tensor(out=kre[:, c4, :], in0=lp[0:32, :],
                                        in1=s1b[0:32, :], op=AL.mult)
                nc.vector.tensor_tensor(out=kro[:, c4, :], in0=lp[32:64, :],
                                        in1=s1b[0:32, :], op=AL.mult)
            nc.vector.tensor_mul(out=krx[0:32], in0=kre[:], in1=cosb_sb[:])
            nc.vector.tensor_mul(out=t2r[:], in0=kro[:], in1=sinb_sb[:])
            nc.vector.tensor_sub(out=krx[0:32], in0=krx[0:32], in1=t2r[:])
            nc.vector.tensor_mul(out=t1r[:], in0=kre[:], in1=sinb_sb[:])
            nc.vector.tensor_mul(out=t2r[:], in0=kro[:], in1=cosb_sb[:])
            nc.vector.tensor_add(out=krx[32:64], in0=t1r[:], in1=t2r[:])

            # ---- A3: q per head (q_up streamed in halves; rope batched x4) ----
            with tc.tile_pool(name="psqp", bufs=3, space="PSUM") as ps_qp:
                for hg in range(2):
                    qup_sb = p_qu.tile([P, 3, 4 * QKD], F32R, tag="qup")
                    nc.sync.dma_start(out=qup_sb[:],
                                      in_=q_up[:, :, hg * 4 * QKD:(hg + 1) * 4 * QKD])
                    qre = p_actA.tile([ROPE // 2, 4, NLOC], BF16, tag="re")
                    qro = p_actA.tile([ROPE // 2, 4, NLOC], BF16, tag="ro")
                    for hh in range(4):
                        h = hg * 4 + hh
                        qn_ps = ps_qp.tile([P, NLOC], F32, tag="qp")
                        for t in range(3):
                            nc.tensor.matmul(qn_ps[:],
                                             qup_sb[:, t, hh * QKD:hh * QKD + NOPE],
                                             qln[:, t, :], start=(t == 0), stop=(t == 2))
                        qn_sb = p_sc.tile([P, NLOC], F32R, tag="qnsb")
                        nc.vector.tensor_copy(out=qn_sb[:], in_=qn_ps[:])
                        for i in range(2):
                            qa_ps = ps_qp.tile([P, NLOC], F32, tag="qp")
                            nc.tensor.matmul(qa_ps[:], wkTT_sb[:, h, i * 128:(i + 1) * 128],
                                             qn_sb[:], start=True, stop=True)
                            nc.vector.tensor_copy(out=qabs[:, 2 * h + i, :], in_=qa_ps[:])
                        qr_ps = ps_qp.tile([P, NLOC], F32, tag="qp")
                        for t in range(3):
                            nc.tensor.matmul(qr_ps[:ROPE],
                                             qup_sb[:, t, hh * QKD + NOPE:(hh + 1) * QKD],
                                             qln[:, t, :], start=(t == 0), stop=(t == 2))
                        nc.scalar.activation(out=qre[:, hh, :], in_=qr_ps[0:32, :],
                                             func=AF.Copy)
                        nc.scalar.activation(out=qro[:, hh, :], in_=qr_ps[32:64, :],
                                             func=AF.Copy)
                    hs = slice(hg * 4, hg * 4 + 4)
                    t1q = p_actA.tile([ROPE // 2, 4, NLOC], BF16, tag="t1r")
                    t2q = p_actA.tile([ROPE // 2, 4, NLOC], BF16, tag="t2r")
                    nc.vector.tensor_mul(out=qrope[0:32, hs, :],
                                         in0=qre[:], in1=cos4_sb[:])
                    nc.vector.tensor_mul(out=t2q[:], in0=qro[:], in1=sin4_sb[:])
                    nc.vector.tensor_sub(out=qrope[0:32, hs, :],
                                         in0=qrope[0:32, hs, :], in1=t2q[:])
                    nc.vector.tensor_mul(out=t1q[:], in0=qre[:], in1=sin4_sb[:])
                    nc.vector.tensor_mul(out=t2q[:], in0=qro[:], in1=cos4_sb[:])
                    nc.vector.tensor_add(out=qrope[32:64, hs, :],
                                         in0=t1q[:], in1=t2q[:])

        g_a.close()   # free xTb, lat weights, qup, wkTT, xb chunks

        # masks + wv (B-scope) and phase-C/D weight prefetch
        g_m = ES()
        p_m = g_m.enter_context(tc.tile_pool(name="pm", bufs=1, side="right"))
        m1_sb = p_m.tile([P, 8, NLOC], BF16, tag="m1")
        nc.sync.dma_start(out=m1_sb[:], in_=mask1.rearrange("a p n -> p a n"))
        m2_sb = p_m.tile([P, 8, 256], BF16, tag="m2")
        nc.sync.dma_start(out=m2_sb[:], in_=mask2.rearrange("a p n -> p a n"))
        wv_sb = p_m.tile([P, 2, H * VD], F32R, tag="wv")
        nc.sync.dma_start(out=wv_sb[:], in_=wv_w[:])
        kvtm = p_m.tile([P, 16, KVLR], F32R, tag="kvtm")
        with tc.tile_pool(name="pstp", bufs=2, space="PSUM") as ps_tp:
            for kt in range(16):
                for dsi in range(2):
                    tp = ps_tp.tile([P, P], F32R, tag="tp")
                    nc.tensor.transpose(tp[:], kvn[:, dsi, kt * 128:(kt + 1) * 128],
                                        identr[:])
                    nc.vector.tensor_copy(
                        out=kvtm[:, kt, dsi * 128:(dsi + 1) * 128], in_=tp[:])

        p_wC1 = ES()
        p_cproj = p_wC1.enter_context(tc.tile_pool(name="pcproj", bufs=1))
        cw_sb = p_cproj.tile([P, 8, D], F32R, tag="cproj")
        nc.gpsimd.dma_start(out=cw_sb[:], in_=cproj_w[:])
        gw_sb = p_cproj.tile([P, 8, E], F32R, tag="gw")
        nc.gpsimd.dma_start(out=gw_sb[:], in_=gate_w[:])

        # ================= phase B: attention =================
        with tc.tile_pool(name="psst", bufs=2, space="PSUM") as ps_st, \
             tc.tile_pool(name="psol", bufs=1, space="PSUM") as ps_ol, \
             tc.tile_pool(name="psden", bufs=1, space="PSUM") as ps_den, \
             tc.tile_pool(name="patt", bufs=2) as p_att:
            for h in range(H):
                olA0 = ps_ol.tile([P, 256], F32, tag="olA0")
                olA1 = ps_ol.tile([P, 256], F32, tag="olA1")
                olB0 = ps_ol.tile([P, 256], F32, tag="olB0")
                olB1 = ps_ol.tile([P, 256], F32, tag="olB1")
                ols = [olA0, olA1, olB0, olB1]
                denA = ps_den.tile([P, 256], F32, tag="denA")
                denB = ps_den.tile([P, 256], F32, tag="denB")
                dens = [denA, denB]
                for kt in range(16):
                    slab1 = kt < 8
                    w = NLOC if slab1 else 256
                    qof = 0 if slab1 else 256
                    kc = slice(kt * 128, (kt + 1) * 128)
                    st = ps_st.tile([P, NLOC], F32, tag="st")
                    nc.tensor.matmul(st[:, :w], kvn[:, 0, kc],
                                     qabs[:, 0 + 2 * h, qof:NLOC], start=True, stop=False)
                    nc.tensor.matmul(st[:, :w], kvn[:, 1, kc],
                                     qabs[:, 1 + 2 * h, qof:NLOC], start=False, stop=False)
                    nc.tensor.matmul(st[:, :w],
                                     krx[:, kt // 4,
                                         (kt % 4) * 128:(kt % 4 + 1) * 128],
                                     qrope[:, h, qof:NLOC], start=False, stop=True)
                    pt = p_att.tile([P, NLOC], F32R, tag="pt")
                    nc.scalar.activation(out=pt[:, :w], in_=st[:, :w], func=AF.Exp)
                    msb = m1_sb[:, kt, :] if slab1 else m2_sb[:, kt - 8, :]
                    nc.vector.tensor_tensor(out=pt[:, :w], in0=pt[:, :w], in1=msb,
                                            op=AL.mult)
                    if slab1:
                        nc.tensor.matmul(denA[:], ones128r[:], pt[:, 0:256],
                                         start=(kt == 0), stop=(kt == 7))
                        for half in range(2):
                            nc.tensor.matmul(ols[half][:],
                                             kvtm[:, kt, half * 128:(half + 1) * 128],
                                             pt[:, 0:256], start=(kt == 0), stop=(kt == 7))
                        pB = pt[:, 256:NLOC]
                    else:
                        pB = pt[:, 0:256]
                    nc.tensor.matmul(denB[:], ones128r[:], pB,
                                     start=(kt == 0), stop=(kt == 15))
                    for half in range(2):
                        nc.tensor.matmul(ols[2 + half][:],
                                         kvtm[:, kt, half * 128:(half + 1) * 128],
                                         pB, start=(kt == 0), stop=(kt == 15))
                for s in range(2):
                    olsb = p_att.tile([P, 2, 256], F32R, tag="olsb")
                    for half in range(2):
                        nc.vector.tensor_copy(out=olsb[:, half, :],
                                              in_=ols[2 * s + half][:])
                    rb = p_att.tile([P, 256], F32, tag="rb")
                    nc.vector.reciprocal(out=rb[:], in_=dens[s][:])
                    yp = ps_st.tile([P, NLOC], F32, tag="st")
                    nc.tensor.matmul(yp[:, 0:256], wv_sb[:, 0, h * VD:(h + 1) * VD],
                                     olsb[:, 0, :], start=True, stop=False)
                    nc.tensor.matmul(yp[:, 0:256], wv_sb[:, 1, h * VD:(h + 1) * VD],
                                     olsb[:, 1, :], start=False, stop=True)
                    nc.vector.tensor_tensor(out=yT[:, h, s * 256:(s + 1) * 256],
                                            in0=yp[:, 0:256], in1=rb[:], op=AL.mult)

        g_m.close()
        g_q.close()
        g_kv.close()

        # ================= phase C: cproj + rms2 + gate =================
        p_wC2 = ES()
        p_sh = p_wC2.enter_context(tc.tile_pool(name="psh", bufs=1))
        sel8_sb = p_sh.tile([E, E * P], BF16, tag="sel8")
        nc.gpsimd.dma_start(out=sel8_sb[:], in_=sel8[:])
        sh1_sb = p_sh.tile([P, 8, INTER * NSH], FP8, tag="sh1")
        nc.gpsimd.dma_start(out=sh1_sb[:], in_=shw1[:])
        sh3_sb = p_sh.tile([P, 8, INTER * NSH], FP8, tag="sh3")
        nc.gpsimd.dma_start(out=sh3_sb[:], in_=shw3[:])
        sh2_sb = p_sh.tile([P, 8, D], BF16, tag="sh2")
        nc.gpsimd.dma_start(out=sh2_sb[:], in_=shw2[:])
        with tc.tile_pool(name="pscC", bufs=2) as p_scC, \
             tc.tile_pool(name="psmm", bufs=3, space="PSUM") as ps_mm, \
             tc.tile_pool(name="psrow2", bufs=1, space="PSUM") as ps_row2, \
             tc.tile_pool(name="pstp2", bufs=2, space="PSUM") as ps_tp2:

            nc.gpsimd.dma_start(out=x2[:], in_=xT.rearrange("(a p) n -> p a n", p=P))
            for ft in range(8):
                op = ps_mm.tile([P, NLOC], F32, tag="mm")
                for ds in range(8):
                    nc.tensor.matmul(op[:], cw_sb[:, ds, ft * 128:(ft + 1) * 128],
                                     yT[:, ds, :], start=(ds == 0), stop=(ds == 7))
                nc.vector.tensor_add(out=x2[:, ft, :], in0=op[:], in1=x2[:, ft, :])

            # rms2 (exact fp32 squares: feeds the gate)
            ss2 = ps_row2.tile([P, NLOC], F32, tag="row2")
            for ds in range(8):
                xsq = p_scC.tile([P, NLOC], F32R, tag="xsq32")
                nc.vector.tensor_mul(out=xsq[:], in0=x2[:, ds, :], in1=x2[:, ds, :])
                nc.tensor.matmul(ss2[:], ones128r[:], xsq[:],
                                 start=(ds == 0), stop=(ds == 7))
            # plain rsqrt for the gate (row 0 used by the transpose below)
            srow2 = p_scC.tile([P, NLOC], F32, tag="srow2")
            nc.scalar.activation(out=srow2[:], in_=ss2[:],
                                 func=AF.Sqrt, bias=eps128[:], scale=1.0 / D)
            nc.vector.reciprocal(out=srow2[:], in_=srow2[:])
            # HS-folded rsqrt for the fp8 activations
            s2b = p_scC.tile([P, NLOC], F32, tag="s2b")
            nc.scalar.activation(out=s2b[:], in_=ss2[:], func=AF.Sqrt,
                                 bias=eps128[:], scale=1.0 / (D * HS * HS))
            nc.vector.reciprocal(out=s2b[:], in_=s2b[:])
            for ds in range(8):
                nc.vector.tensor_tensor(out=h2f8[:, ds, :], in0=x2[:, ds, :],
                                        in1=s2b[:], op=AL.mult)

            # gate
            gp = ps_mm.tile([P, NLOC], F32, tag="mm")
            for ds in range(8):
                nc.tensor.matmul(gp[:E], gw_sb[:, ds, :], x2[:, ds, :],
                                 start=(ds == 0), stop=(ds == 7))
            g_sb = p_scC.tile([E, NLOC], F32, tag="gsb")
            nc.vector.tensor_copy(out=g_sb[:], in_=gp[:E])
            for q4 in range(4):
                tp = ps_tp2.tile([P, P], F32, tag="tp2")
                nc.tensor.transpose(tp[:, 0:E], g_sb[:, q4 * 128:(q4 + 1) * 128],
                                    identf[0:E, 0:E])
                gt = p_scC.tile([P, E], F32, tag="gt")
                nc.vector.tensor_copy(out=gt[:], in_=tp[:, 0:E])
                s2tp = ps_tp2.tile([P, P], F32, tag="tp2")
                nc.tensor.transpose(s2tp[:, 0:1], srow2[0:1, q4 * 128:(q4 + 1) * 128],
                                    identf[0:1, 0:1])
                s2c = p_scC.tile([P, 1], F32, tag="s2c")
                nc.vector.tensor_copy(out=s2c[:], in_=s2tp[:, 0:1])
                nc.vector.tensor_scalar_mul(out=gt[:], in0=gt[:], scalar1=s2c[:])
                mx = p_scC.tile([P, 4], F32, tag="mx")
                nc.vector.tensor_reduce(out=mx[:, 0:1], in_=gt[:],
                                        axis=mybir.AxisListType.X, op=AL.max)
                nc.vector.tensor_scalar_mul(out=mx[:, 1:2], in0=mx[:, 0:1], scalar1=-1.0)
                e8 = p_scC.tile([P, E], F32, tag="e8")
                nc.scalar.activation(out=e8[:], in_=gt[:], func=AF.Exp,
                                     bias=mx[:, 1:2], accum_out=mx[:, 2:3])
                nc.vector.reciprocal(out=mx[:, 3:4], in_=mx[:, 2:3])
                srt = p_scC.tile([P, E], F32, tag="srt")
                nc.vector.max(out=srt[:], in_=e8[:])
                cwq = p_scC.tile([P, E], F32, tag="cwq")
                nc.vector.tensor_scalar(out=cwq[:], in0=e8[:], scalar1=srt[:, 1:2],
                                        scalar2=None, op0=AL.is_ge)
                nc.vector.tensor_mul(out=cwq[:], in0=cwq[:], in1=e8[:])
                nc.vector.tensor_scalar_mul(out=cwq[:], in0=cwq[:], scalar1=mx[:, 3:4])
                tp2 = ps_tp2.tile([P, P], F32, tag="tp2")
                nc.tensor.transpose(tp2[0:E, 0:P], cwq[:], identf[:])
                nc.vector.tensor_copy(out=cwT[:, q4 * 128:(q4 + 1) * 128],
                                      in_=tp2[0:E, 0:P])

        g_yT.close()

        # ================= phase D: MoE (fp8 DoubleRow, 2-pass) =================
        with tc.tile_pool(name="pscD", bufs=2) as p_scD, \
             tc.tile_pool(name="pwe2", bufs=2) as p_we2, \
             tc.tile_pool(name="phsh", bufs=1) as p_hsh:

            hshs = []
            cwb_sh = p_scD.tile([P, NLOC], F32, tag="cwbsh")
            nc.vector.memset(cwb_sh[:], float(1.0 / QW))

            def mlp13(w1s, w3s, hsh8, nft, cwb):
                for ft in range(nft):
                    g1 = ps_g.tile([P, NLOC], F32, tag="g1")
                    g3 = ps_g.tile([P, NLOC], F32, tag="g3")
                    for dp in range(4):
                        nc.tensor.matmul(g1[:], w1s[:, 2 * dp:2 * dp + 2,
                                                    ft * 128:(ft + 1) * 128],
                                         h2f8[:, 2 * dp:2 * dp + 2, :],
                                         perf_mode=DR, start=(dp == 0), stop=(dp == 3))
                    for dp in range(4):
                        nc.tensor.matmul(g3[:], w3s[:, 2 * dp:2 * dp + 2,
                                                    ft * 128:(ft + 1) * 128],
                                         h2f8[:, 2 * dp:2 * dp + 2, :],
                                         perf_mode=DR, start=(dp == 0), stop=(dp == 3))
                    sl = p_scD.tile([P, NLOC], F32, tag="silu")
                    nc.scalar.activation(out=sl[:], in_=g1[:], func=AF.Silu,
                                         scale=float(1.0 / (HS * QW)))
                    tmp = p_scD.tile([P, NLOC], F32, tag="tmp")
                    nc.vector.tensor_mul(out=tmp[:], in0=sl[:], in1=g3[:])
                    nc.vector.tensor_tensor(out=hsh8[:, ft, :], in0=tmp[:],
                                            in1=cwb[:], op=AL.mult)

            # pass 1: routed experts
            p1 = ES()
            p_we = p1.enter_context(tc.tile_pool(name="pwe", bufs=2))
            ps_g = p1.enter_context(tc.tile_pool(name="psg", bufs=2, space="PSUM"))
            ps_bcd = p1.enter_context(tc.tile_pool(name="psbcd", bufs=2, space="PSUM"))
            for e in range(E):
                e1_sb = p_we.tile([P, 8, INTER], FP8, tag="we1")
                nc.sync.dma_start(out=e1_sb[:], in_=ew1[e])
                e3_sb = p_we.tile([P, 8, INTER], FP8, tag="we3")
                nc.sync.dma_start(out=e3_sb[:], in_=ew3[e])
                hsh8 = p_hsh.tile([P, 4, NLOC], BF16, tag=f"hsh{e}")
                hshs.append(hsh8)
                cwp = ps_bcd.tile([P, NLOC], F32, tag="bc")
                nc.tensor.matmul(cwp[:], sel8_sb[:, e * P:(e + 1) * P], cwT[:],
                                 start=True, stop=True)
                cwb = p_scD.tile([P, NLOC], F32, tag="cwb")
                nc.vector.tensor_copy(out=cwb[:], in_=cwp[:])
                mlp13(e1_sb, e3_sb, hsh8, 4, cwb)
            # pass 1b: shared expert (cw = 1)
            hsh_sh = p_hsh.tile([P, 8, NLOC], BF16, tag="hshsh")
            mlp13(sh1_sb, sh3_sb, hsh_sh, 8, cwb_sh)

            p1.close()
            # pass 2: w2 accumulation across experts + shared, then residual
            with tc.tile_pool(name="psacc", bufs=1, space="PSUM") as ps_acc:
                acc = ps_acc.tile([P, 8, NLOC], F32, tag="acc")
                for e in range(E):
                    e2_sb = p_we2.tile([P, 4, D], BF16, tag="we2")
                    nc.sync.dma_start(out=e2_sb[:], in_=ew2[e])
                    for ft in range(8):
                        for ds in range(4):
                            nc.tensor.matmul(acc[:, ft, :],
                                             e2_sb[:, ds, ft * 128:(ft + 1) * 128],
                                             hshs[e][:, ds, :],
                                             start=(e == 0 and ds == 0), stop=False)
                for ft in range(8):
                    for ds in range(8):
                        nc.tensor.matmul(acc[:, ft, :],
                                         sh2_sb[:, ds, ft * 128:(ft + 1) * 128],
                                         hsh_sh[:, ds, :],
                                         start=False, stop=(ds == 7))
                for ft in range(8):
                    msb = p_scD.tile([P, NLOC], F32, tag="msb")
                    nc.scalar.activation(out=msb[:], in_=acc[:, ft, :], func=AF.Copy,
                                         scale=float(1.0 / (HS * QW)))
                    nc.vector.tensor_add(out=x2[:, ft, :], in0=msb[:], in1=x2[:, ft, :])

            nc.sync.dma_start(out=out_xT.rearrange("(a p) n -> p a n", p=P), in_=x2[:])

        p_wC2.close()
        p_wC1.close()
        g_x.close()

    nc.finalize()
    return nc


# ============================ host side ============================

_CACHE = {}


def _prep_shared(inputs):
    perm = _rope_perm()
    f = np.float32
    latent_w = (np.asarray(inputs["latent_w"], f)
                * np.asarray(inputs["rmsn1_w"], f)[:, None]).copy()
    latent_w[:, QLR + KVLR:] = latent_w[:, QLR + KVLR:][:, perm]
    q_up = (np.asarray(inputs["q_up_w"], f)
            * np.asarray(inputs["q_norm_w"], f)[:, None]).copy()
    for h in range(H):
        c0 = h * QKD + NOPE
        q_up[:, c0:c0 + ROPE] = q_up[:, c0:c0 + ROPE][:, perm]
    q_up *= SCALE
    kv_up = (np.asarray(inputs["kv_up_w"], f)
             * np.asarray(inputs["kv_norm_w"], f)[:, None])
    wk = np.stack([kv_up[:, h * (NOPE + VD):h * (NOPE + VD) + NOPE].T
                   for h in range(H)], axis=1)   # [NOPE, H, KVLR]
    wv = np.concatenate([kv_up[:, h * (NOPE + VD) + NOPE:(h + 1) * (NOPE + VD)]
                         for h in range(H)], axis=1)
    r2 = np.asarray(inputs["rmsn2_w"], f)[:, None]
    latw_t = _tile_w(latent_w)           # [P, 8, 704]
    shared = {
        "ident": np.eye(P, dtype=f),
        "latq_w": np.ascontiguousarray(latw_t[:, :, :QLR]).astype(BFNP),
        "latkv_w": np.ascontiguousarray(latw_t[:, :, QLR:]).astype(BFNP),
        "q_up": _tile_w(q_up),
        "wkTT": np.ascontiguousarray(wk.astype(f)),
        "wv_w": _tile_w(wv.astype(f)),
        "cproj_w": _tile_w(np.asarray(inputs["c_proj_w"], f)),
        "gate_w": _tile_w(np.asarray(inputs["gate_w"], f) * r2),
        "shw1": _f8(_tile_w(np.asarray(inputs["sh_w1"], f) * r2) * QW),
        "shw3": _f8(_tile_w(np.asarray(inputs["sh_w3"], f) * r2) * QW),
        "shw2": (_tile_w(np.asarray(inputs["sh_w2"], f)) * QW).astype(BFNP),
        "ew1": np.stack([_f8(_tile_w(np.asarray(inputs["e_w1"], f)[e] * r2) * QW)
                         for e in range(E)]),
        "ew3": np.stack([_f8(_tile_w(np.asarray(inputs["e_w3"], f)[e] * r2) * QW)
                         for e in range(E)]),
        "ew2": np.stack([(_tile_w(np.asarray(inputs["e_w2"], f)[e]) * QW).astype(BFNP)
                         for e in range(E)]),
        "sel8": np.repeat(np.eye(E, dtype=f) * (1.0 / QW), P, axis=1)
                  .reshape(E, E * P).astype(BFNP),
    }
    return shared


def _prep_core(inputs, c):
    f = np.float32
    pos = _core_positions(c)
    b = c // 4
    gidx = b * T + pos
    xflat = np.asarray(inputs["x"], dtype=f).reshape(N, D)
    xT_c = np.ascontiguousarray(xflat[gidx].T)
    xTb_c = np.ascontiguousarray(
        xflat[gidx].reshape(NLOC, 8, P).transpose(2, 1, 0)).astype(BFNP)
    xb_c = np.ascontiguousarray(
        xflat[b * T:(b + 1) * T].reshape(T, 8, P).transpose(2, 1, 0)
    ).astype(BFNP)
    cosb = np.ascontiguousarray(np.asarray(inputs["freqs_cos"], f).T
                                ).reshape(ROPE // 2, 4, 512).astype(BFNP)
    sinb = np.ascontiguousarray(np.asarray(inputs["freqs_sin"], f).T
                                ).reshape(ROPE // 2, 4, 512).astype(BFNP)
    cosT = np.asarray(inputs["freqs_cos"], f)[pos].T   # [32, NLOC]
    sinT = np.asarray(inputs["freqs_sin"], f)[pos].T
    cosT4 = np.ascontiguousarray(
        np.repeat(cosT[:, None, :], 4, axis=1)).astype(BFNP)
    sinT4 = np.ascontiguousarray(
        np.repeat(sinT[:, None, :], 4, axis=1)).astype(BFNP)
    k_abs = (np.arange(8)[:, None] * 128 + np.arange(P)[None, :])
    m1 = np.where(k_abs[:, :, None] <= pos[None, None, :], 1.0, 0.0)
    k_abs2 = ((np.arange(8, 16))[:, None] * 128 + np.arange(P)[None, :])
    m2 = np.where(k_abs2[:, :, None] <= pos[None, None, 256:], 1.0, 0.0)
    return {
        "xT": xT_c, "xTb": xTb_c, "xb": xb_c, "cosbT": cosb, "sinbT": sinb,
        "cosT4": cosT4, "sinT4": sinT4,
        "mask1": m1.astype(BFNP),
        "mask2": m2.astype(BFNP),
    }, gidx


def run(inputs, trace=False, **kw):
    if "nc" not in _CACHE:
        _CACHE["nc"] = build()
    nc = _CACHE["nc"]
    shared = _prep_shared(inputs)
    in_maps = []
    gidxs = []
    for c in range(NCORES):
        m, gidx = _prep_core(inputs, c)
        m.update(shared)
        in_maps.append(m)
        gidxs.append(gidx)
    res = run_bass_kernel_spmd(nc, in_maps, core_ids=list(range(NCORES)),
                               trace=trace, **kw)
    full = np.empty((N, D), dtype=np.float32)
    for c in range(NCORES):
        full[gidxs[c]] = np.asarray(res.results[c]["out_xT"], np.float32).T
    return full.reshape(B, T, D), res


def kernel(**inputs):
    out, _ = run(inputs)
    return out


# revision 30
# speedup vs baseline: 1.3676x; 1.1546x over previous
"""MLA + DeepSeekMoE block on 8 trn2 NeuronCores (Bass/Tile SPMD).

Token-sharded across 8 cores (512 tokens each, causally balanced stripes).
v2: no collective -- each core recomputes the KV latent (kv256+rope64) for
all 2048 batch tokens from a bf16 copy of x (MLA absorption trick).
Attention operands in bf16 (same PE rate as fp32r, half the SBUF/DVE).
MoE (shared + all 8 routed experts, computed densely) in fp8-e4m3 with
DoubleRow matmuls; expert w2 outputs accumulate across experts in PSUM.
cproj/gate stay fp32r so the gate top-2 selection matches the reference.

v3 perf changes (numerics-preserving):
  - removed all debug outputs (10+ MB of HBM writes at phase boundaries)
  - all row stats (rms / softmax denom) computed as [128, N] via all-ones
    [128,128] stationary matmuls: kills 1-partition DVE ops (3.3us each),
    the brow broadcast matmul+copy, and the slow M=1 matmuls
  - causal mask applied multiplicatively after exp (shorter PSUM chain)
  - rope rotation batched across heads (A3) / chunks (A2) on full tiles
  - squares for rms stats moved to the idle Scalar engine (Square act)
  - softmax scale folded into q_up on the host; attention output
    normalized after the wv matmul (one less DVE op per head-slab)
"""

import numpy as np
import ml_dtypes

import concourse.bacc as bacc
import concourse.mybir as mybir
from concourse.tile import TileContext
from concourse.bass_utils import run_bass_kernel_spmd

# ---- problem constants ----
D = 1024; H = 8; QLR = 384; KVLR = 256; NOPE = 128; ROPE = 64; VD = 128
E = 8; TOPK = 2; INTER = 512; NSH = 2; B = 2; T = 2048; QKD = NOPE + ROPE
N = B * T
NCORES = 8
NLOC = N // NCORES          # 512 tokens per core
P = 128
EPS = 1e-6
SCALE = 1.0 / np.sqrt(QKD)
QW = 64.0                   # fp8 weight scale
HS = 16.0                   # fp8 h2 / hsh scale

F32 = mybir.dt.float32
F32R = mybir.dt.float32r
BF16 = mybir.dt.bfloat16
FP8 = mybir.dt.float8e4
DR = mybir.MatmulPerfMode.DoubleRow
E4NP = ml_dtypes.float8_e4m3
BFNP = ml_dtypes.bfloat16

# MoE fp8 scale plan:
#   h2f8 = HS*h2;  w1,w3,w2 fp8 x QW
#   g1 psum = HS*QW*g1_true -> silu(scale=1/(HS*QW)) = silu_true
#   tmp = silu*g3 = HS*QW*hsh_true
#   hsh8 = tmp * cwb,  cwb = cw/QW  (sel8 prescaled 1/QW; shared cw=1)
#   acc psum = hsh8 @ w2(xQW) = HS*QW*(cw-weighted moe) -> copy scale 1/(HS*QW)


def _rope_perm():
    return np.concatenate([np.arange(0, ROPE, 2), np.arange(1, ROPE, 2)])


def _core_positions(c):
    j = c % 4
    return np.concatenate([np.arange(j * 256, (j + 1) * 256),
                           np.arange((7 - j) * 256, (8 - j) * 256)])


def _tile_w(w):
    """[K, F] row-major -> [128, K//128, F] partition-major contiguous."""
    K, F = w.shape
    return np.ascontiguousarray(w.reshape(K // P, P, F).transpose(1, 0, 2))


def _f8(a):
    return np.ascontiguousarray(np.asarray(a, np.float32)).astype(E4NP)


# ============================ device program ============================

def build():
    from contextlib import ExitStack as ES
    nc = bacc.Bacc(name="mla_moe_v3")

    # ---- I/O ----
    xT = nc.dram_tensor("xT", [D, NLOC], F32R, kind="ExternalInput")
    xTb = nc.dram_tensor("xTb", [P, 8, NLOC], BF16, kind="ExternalInput")
    xb = nc.dram_tensor("xb", [P, 8, T], BF16, kind="ExternalInput")
    cosbT = nc.dram_tensor("cosbT", [ROPE // 2, 4, 512], BF16, kind="ExternalInput")
    sinbT = nc.dram_tensor("sinbT", [ROPE // 2, 4, 512], BF16, kind="ExternalInput")
    cosT4 = nc.dram_tensor("cosT4", [ROPE // 2, 4, NLOC], BF16, kind="ExternalInput")
    sinT4 = nc.dram_tensor("sinT4", [ROPE // 2, 4, NLOC], BF16, kind="ExternalInput")
    mask1 = nc.dram_tensor("mask1", [P, 8, NLOC], BF16, kind="ExternalInput")
    mask2 = nc.dram_tensor("mask2", [P, 8, 256], BF16, kind="ExternalInput")
    ident = nc.dram_tensor("ident", [P, P], F32, kind="ExternalInput")
    latq_w = nc.dram_tensor("latq_w", [P, 8, QLR], BF16, kind="ExternalInput")
    latkv_w = nc.dram_tensor("latkv_w", [P, 8, KVLR + 128], BF16, kind="ExternalInput")
    q_up = nc.dram_tensor("q_up", [P, 3, H * 256], F32R, kind="ExternalInput")
    wkTT = nc.dram_tensor("wkTT", [P, H, KVLR], F32R, kind="ExternalInput")
    wv_w = nc.dram_tensor("wv_w", [P, 2, H * VD], F32R, kind="ExternalInput")
    cproj_w = nc.dram_tensor("cproj_w", [P, 8, D], F32R, kind="ExternalInput")
    gate_w = nc.dram_tensor("gate_w", [P, 8, E], F32R, kind="ExternalInput")
    shw1 = nc.dram_tensor("shw1", [P, 8, INTER * NSH], FP8, kind="ExternalInput")
    shw3 = nc.dram_tensor("shw3", [P, 8, INTER * NSH], FP8, kind="ExternalInput")
    shw2 = nc.dram_tensor("shw2", [P, 8, D], BF16, kind="ExternalInput")
    ew1 = nc.dram_tensor("ew1", [E, P, 8, INTER], FP8, kind="ExternalInput")
    ew3 = nc.dram_tensor("ew3", [E, P, 8, INTER], FP8, kind="ExternalInput")
    ew2 = nc.dram_tensor("ew2", [E, P, 4, D], BF16, kind="ExternalInput")
    sel8 = nc.dram_tensor("sel8", [E, E * P], BF16, kind="ExternalInput")
    out_xT = nc.dram_tensor("out_xT", [P, 8, NLOC], F32R, kind="ExternalOutput")

    AL = mybir.AluOpType
    AF = mybir.ActivationFunctionType

    with TileContext(nc) as tc, \
         nc.allow_low_precision(reason="f32r rows / bf16+fp8 activations by design"), \
         tc.tile_pool(name="const", bufs=1) as p_const, \
         tc.tile_pool(name="psc", bufs=2) as p_sc:

        # right-side LIFO stack (open order = reverse close order)
        g_x = ES();  p_x = g_x.enter_context(tc.tile_pool(name="px", bufs=1, side="right"))
        g_yT = ES(); p_yt = g_yT.enter_context(tc.tile_pool(name="pyt", bufs=1, side="right"))
        g_kv = ES(); p_kv = g_kv.enter_context(tc.tile_pool(name="pkv", bufs=1, side="right"))
        g_q = ES();  p_q = g_q.enter_context(tc.tile_pool(name="pq", bufs=1, side="right"))
        g_a = ES()
        p_wA = g_a.enter_context(tc.tile_pool(name="pwA", bufs=1, side="right"))
        p_qu = g_a.enter_context(tc.tile_pool(name="pqu", bufs=1, side="right"))
        p_xb = g_a.enter_context(tc.tile_pool(name="pxb", bufs=2, side="right"))

        # ---- constants ----
        identf = p_const.tile([P, P], F32, tag="identf")
        nc.gpsimd.dma_start(out=identf[:], in_=ident[:])
        identr = p_const.tile([P, P], F32R, tag="identr")
        nc.vector.tensor_copy(out=identr[:], in_=identf[:])
        ones128f = p_const.tile([P, P], F32, tag="ones128f")
        nc.vector.memset(ones128f[:], 1.0)
        ones128b = p_const.tile([P, P], BF16, tag="ones128b")
        nc.vector.tensor_copy(out=ones128b[:], in_=ones128f[:])
        ones128r = p_const.tile([P, P], F32R, tag="ones128r")
        nc.vector.tensor_copy(out=ones128r[:], in_=ones128f[:])
        eps128 = p_const.tile([P, 1], F32, tag="eps128")
        nc.vector.memset(eps128[:], EPS)


        # ---- persistent activations ----
        x2 = p_x.tile([P, 8, NLOC], F32R, tag="x2")
        h2f8 = p_x.tile([P, 8, NLOC], FP8, tag="h2f8")
        cwT = p_x.tile([E, NLOC], BF16, tag="cwT")
        yT = p_yt.tile([P, H, NLOC], F32R, tag="yT")

        kvn = p_kv.tile([P, 2, T], F32R, tag="kvn")
        krx = p_kv.tile([P, 4, 512], BF16, tag="krx")
        nc.vector.memset(krx[ROPE:P], 0.0)

        qabs = p_q.tile([P, 2 * H, NLOC], F32R, tag="qabs")
        qrope = p_q.tile([P, H, NLOC], BF16, tag="qrope")
        nc.vector.memset(qrope[ROPE:P], 0.0)

        cos4_sb = p_wA.tile([ROPE // 2, 4, NLOC], BF16, tag="cos4")
        nc.gpsimd.dma_start(out=cos4_sb[:], in_=cosT4[:])
        sin4_sb = p_wA.tile([ROPE // 2, 4, NLOC], BF16, tag="sin4")
        nc.gpsimd.dma_start(out=sin4_sb[:], in_=sinT4[:])
        cosb_sb = p_wA.tile([ROPE // 2, 4, 512], BF16, tag="cosb")
        nc.gpsimd.dma_start(out=cosb_sb[:], in_=cosbT[:])
        sinb_sb = p_wA.tile([ROPE // 2, 4, 512], BF16, tag="sinb")
        nc.gpsimd.dma_start(out=sinb_sb[:], in_=sinbT[:])

        xTb_sb = p_wA.tile([P, 8, NLOC], BF16, tag="xTb")
        nc.sync.dma_start(out=xTb_sb[:], in_=xTb[:])
        latq_sb = p_wA.tile([P, 8, QLR], BF16, tag="latqw")
        nc.sync.dma_start(out=latq_sb[:], in_=latq_w[:])
        latkv_sb = p_wA.tile([P, 8, KVLR + 128], BF16, tag="latkvw")
        nc.sync.dma_start(out=latkv_sb[:], in_=latkv_w[:])
        wkTT_sb = p_wA.tile([P, H, KVLR], F32R, tag="wkTT")
        nc.sync.dma_start(out=wkTT_sb[:], in_=wkTT[:])

        def rstat(row_ps, scale):
            """[128,N] PSUM sum-of-squares -> [128,N] SBUF rsqrt(mean+eps)."""
            w = row_ps.shape[-1]
            sb = p_sc.tile([P, w], F32, tag="rstat")
            nc.scalar.activation(out=sb[:], in_=row_ps[:],
                                 func=mybir.ActivationFunctionType.Sqrt,
                                 bias=eps128[:], scale=scale)
            nc.vector.reciprocal_approx_fast(out=sb[:], in_=sb[:])
            return sb

        # ================= phase A =================
        with tc.tile_pool(name="actA", bufs=1) as p_actA, \
             tc.tile_pool(name="pr", bufs=1) as p_r, \
             tc.tile_pool(name="pslat", bufs=2, space="PSUM") as ps_lat, \
             tc.tile_pool(name="psrow", bufs=2, space="PSUM") as ps_row:

            # ---- A1: local rms1 + q latents (bf16 inputs; scores-only) ----
            ss_ps = ps_row.tile([P, NLOC], F32, tag="ss")
            for ds in range(8):
                xsq = p_sc.tile([P, NLOC], BF16, tag="xsq")
                nc.scalar.activation(out=xsq[:], in_=xTb_sb[:, ds, :], func=AF.Square)
                nc.tensor.matmul(ss_ps[:], ones128b[:], xsq[:],
                                 start=(ds == 0), stop=(ds == 7))
            s1loc = rstat(ss_ps, 1.0 / D)

            qln = p_actA.tile([P, 3, NLOC], F32R, tag="qln")
            for ft in range(3):
                lp = ps_lat.tile([P, NLOC], F32, tag="lat")
                for ds in range(8):
                    nc.tensor.matmul(lp[:], latq_sb[:, ds, ft * 128:(ft + 1) * 128],
                                     xTb_sb[:, ds, :], start=(ds == 0), stop=(ds == 7))
                nc.vector.tensor_tensor(out=qln[:, ft, :], in0=lp[:],
                                        in1=s1loc[:], op=AL.mult)
            sq_ps = ps_row.tile([P, NLOC], F32, tag="ss")
            for t in range(3):
                xsq = p_sc.tile([P, NLOC], BF16, tag="xsq")
                nc.scalar.activation(out=xsq[:], in_=qln[:, t, :], func=AF.Square)
                nc.tensor.matmul(sq_ps[:], ones128b[:], xsq[:],
                                 start=(t == 0), stop=(t == 2))
            sqb = rstat(sq_ps, 1.0 / QLR)
            for t in range(3):
                nc.vector.tensor_tensor(out=qln[:, t, :], in0=qln[:, t, :],
                                        in1=sqb[:], op=AL.mult)

            # ---- A2: batch kv latents, 4 chunks of 512 tokens ----
            kre = p_actA.tile([ROPE // 2, 4, 512], BF16, tag="re")
            kro = p_actA.tile([ROPE // 2, 4, 512], BF16, tag="ro")
            t1r = p_actA.tile([ROPE // 2, 4, 512], BF16, tag="t1r")
            t2r = p_actA.tile([ROPE // 2, 4, 512], BF16, tag="t2r")
            for c4 in range(4):
                cs = slice(c4 * 512, (c4 + 1) * 512)
                xbc = p_xb.tile([P, 8, NLOC], BF16, tag="xbc")
                nc.sync.dma_start(out=xbc[:], in_=xb[:, :, cs])
                ssb = ps_row.tile([P, NLOC], F32, tag="ss")
                for ds in range(8):
                    xsq = p_sc.tile([P, NLOC], BF16, tag="xsq")
                    nc.scalar.activation(out=xsq[:], in_=xbc[:, ds, :], func=AF.Square)
                    nc.tensor.matmul(ssb[:], ones128b[:], xsq[:],
                                     start=(ds == 0), stop=(ds == 7))
                s1b = rstat(ssb, 1.0 / D)

                kvt = p_r.tile([P, 2, NLOC], F32, tag="kvt")
                for i in range(2):
                    lp = ps_lat.tile([P, NLOC], F32, tag="lat")
                    for ds in range(8):
                        nc.tensor.matmul(lp[:], latkv_sb[:, ds, i * 128:(i + 1) * 128],
                                         xbc[:, ds, :], start=(ds == 0), stop=(ds == 7))
                    nc.vector.tensor_tensor(out=kvt[:, i, :], in0=lp[:],
                                            in1=s1b[:], op=AL.mult)
                kss = ps_row.tile([P, NLOC], F32, tag="ss")
                for i in range(2):
                    xsq = p_sc.tile([P, NLOC], BF16, tag="xsq")
                    nc.scalar.activation(out=xsq[:], in_=kvt[:, i, :], func=AF.Square)
                    nc.tensor.matmul(kss[:], ones128b[:], xsq[:],
                                     start=(i == 0), stop=(i == 1))
                skvb = rstat(kss, 1.0 / KVLR)
                for i in range(2):
                    nc.vector.tensor_tensor(out=kvn[:, i, cs], in0=kvt[:, i, :],
                                            in1=skvb[:], op=AL.mult)
                # rope rows x s1 -> staged; rotation batched after the loop
                lp = ps_lat.tile([P, NLOC], F32, tag="lat")
                for ds in range(8):
                    nc.tensor.matmul(lp[:], latkv_sb[:, ds, KVLR:KVLR + 128],
                                     xbc[:, ds, :], start=(ds == 0), stop=(ds == 7))
                nc.vector.tensor_tensor(out=kre[:, c4, :], in0=lp[0:32, :],
                                        in1=s1b[0:32, :], op=AL.mult)
                nc.vector.tensor_tensor(out=kro[:, c4, :], in0=lp[32:64, :],
                                        in1=s1b[0:32, :], op=AL.mult)
            nc.vector.tensor_mul(out=krx[0:32], in0=kre[:], in1=cosb_sb[:])
            nc.vector.tensor_mul(out=t2r[:], in0=kro[:], in1=sinb_sb[:])
            nc.vector.tensor_sub(out=krx[0:32], in0=krx[0:32], in1=t2r[:])
            nc.vector.tensor_mul(out=t1r[:], in0=kre[:], in1=sinb_sb[:])
            nc.vector.tensor_mul(out=t2r[:], in0=kro[:], in1=cosb_sb[:])
            nc.vector.tensor_add(out=krx[32:64], in0=t1r[:], in1=t2r[:])

            # ---- A3: q per head (q_up streamed in halves; rope batched x4) ----
            with tc.tile_pool(name="psqp", bufs=3, space="PSUM") as ps_qp:
                for hg in range(2):
                    qup_sb = p_qu.tile([P, 3, 4 * 256], F32R, tag="qup")
                    nc.sync.dma_start(out=qup_sb[:],
                                      in_=q_up[:, :, hg * 1024:(hg + 1) * 1024])
                    qre = p_actA.tile([ROPE // 2, 4, NLOC], BF16, tag="re")
                    qro = p_actA.tile([ROPE // 2, 4, NLOC], BF16, tag="ro")
                    for hh in range(4):
                        h = hg * 4 + hh
                        qn_ps = ps_qp.tile([P, NLOC], F32, tag="qp")
                        for t in range(3):
                            nc.tensor.matmul(qn_ps[:],
                                             qup_sb[:, t, hh * 256:hh * 256 + NOPE],
                                             qln[:, t, :], start=(t == 0), stop=(t == 2))
                        qn_sb = p_sc.tile([P, NLOC], F32R, tag="qnsb")
                        nc.vector.tensor_copy(out=qn_sb[:], in_=qn_ps[:])
                        for i in range(2):
                            qa_ps = ps_qp.tile([P, NLOC], F32, tag="qp")
                            nc.tensor.matmul(qa_ps[:], wkTT_sb[:, h, i * 128:(i + 1) * 128],
                                             qn_sb[:], start=True, stop=True)
                            nc.vector.tensor_copy(out=qabs[:, 2 * h + i, :], in_=qa_ps[:])
                        qr_ps = ps_qp.tile([P, NLOC], F32, tag="qp")
                        for t in range(3):
                            nc.tensor.matmul(qr_ps[:],
                                             qup_sb[:, t, hh * 256 + 128:(hh + 1) * 256],
                                             qln[:, t, :], start=(t == 0), stop=(t == 2))
                        nc.scalar.activation(out=qre[:, hh, :], in_=qr_ps[0:32, :],
                                             func=AF.Copy)
                        nc.scalar.activation(out=qro[:, hh, :], in_=qr_ps[32:64, :],
                                             func=AF.Copy)
                    hs = slice(hg * 4, hg * 4 + 4)
                    t1q = p_actA.tile([ROPE // 2, 4, NLOC], BF16, tag="t1r")
                    t2q = p_actA.tile([ROPE // 2, 4, NLOC], BF16, tag="t2r")
                    nc.vector.tensor_mul(out=qrope[0:32, hs, :],
                                         in0=qre[:], in1=cos4_sb[:])
                    nc.vector.tensor_mul(out=t2q[:], in0=qro[:], in1=sin4_sb[:])
                    nc.vector.tensor_sub(out=qrope[0:32, hs, :],
                                         in0=qrope[0:32, hs, :], in1=t2q[:])
                    nc.vector.tensor_mul(out=t1q[:], in0=qre[:], in1=sin4_sb[:])
                    nc.vector.tensor_mul(out=t2q[:], in0=qro[:], in1=cos4_sb[:])
                    nc.vector.tensor_add(out=qrope[32:64, hs, :],
                                         in0=t1q[:], in1=t2q[:])

        g_a.close()   # free xTb, lat weights, qup, wkTT, xb chunks

        # masks + wv (B-scope) and phase-C/D weight prefetch
        g_m = ES()
        p_m = g_m.enter_context(tc.tile_pool(name="pm", bufs=1, side="right"))
        m1_sb = p_m.tile([P, 8, NLOC], BF16, tag="m1")
        nc.sync.dma_start(out=m1_sb[:], in_=mask1[:])
        m2_sb = p_m.tile([P, 8, 256], BF16, tag="m2")
        nc.sync.dma_start(out=m2_sb[:], in_=mask2[:])
        wv_sb = p_m.tile([P, 2, H * VD], F32R, tag="wv")
        nc.sync.dma_start(out=wv_sb[:], in_=wv_w[:])
        kvtm = p_m.tile([P, 16, KVLR], F32R, tag="kvtm")
        with tc.tile_pool(name="pstp", bufs=2, space="PSUM") as ps_tp:
            for kt in range(16):
                for dsi in range(2):
                    tp = ps_tp.tile([P, P], F32R, tag="tp")
                    nc.tensor.transpose(tp[:], kvn[:, dsi, kt * 128:(kt + 1) * 128],
                                        identr[:])
                    nc.scalar.activation(
                        out=kvtm[:, kt, dsi * 128:(dsi + 1) * 128], in_=tp[:],
                        func=AF.Copy)

        p_wC1 = ES()
        p_cproj = p_wC1.enter_context(tc.tile_pool(name="pcproj", bufs=1))
        cw_sb = p_cproj.tile([P, 8, D], F32R, tag="cproj")
        nc.gpsimd.dma_start(out=cw_sb[:], in_=cproj_w[:])
        gw_sb = p_cproj.tile([P, 8, E], F32R, tag="gw")
        nc.gpsimd.dma_start(out=gw_sb[:], in_=gate_w[:])

        # ================= phase B: attention =================
        with tc.tile_pool(name="psst", bufs=2, space="PSUM") as ps_st, \
             tc.tile_pool(name="psol", bufs=1, space="PSUM") as ps_ol, \
             tc.tile_pool(name="psden", bufs=1, space="PSUM") as ps_den, \
             tc.tile_pool(name="patt", bufs=2) as p_att:
            for h in range(H):
                olA0 = ps_ol.tile([P, 256], F32, tag="olA0")
                olA1 = ps_ol.tile([P, 256], F32, tag="olA1")
                olB0 = ps_ol.tile([P, 256], F32, tag="olB0")
                olB1 = ps_ol.tile([P, 256], F32, tag="olB1")
                ols = [olA0, olA1, olB0, olB1]
                denA = ps_den.tile([P, 256], F32, tag="denA")
                denB = ps_den.tile([P, 256], F32, tag="denB")
                dens = [denA, denB]
                for kt in range(16):
                    slab1 = kt < 8
                    w = NLOC if slab1 else 256
                    qof = 0 if slab1 else 256
                    kc = slice(kt * 128, (kt + 1) * 128)
                    st = ps_st.tile([P, NLOC], F32, tag="st")
                    nc.tensor.matmul(st[:, :w], kvn[:, 0, kc],
                                     qabs[:, 0 + 2 * h, qof:NLOC], start=True, stop=False)
                    nc.tensor.matmul(st[:, :w], kvn[:, 1, kc],
                                     qabs[:, 1 + 2 * h, qof:NLOC], start=False, stop=False)
                    nc.tensor.matmul(st[:, :w],
                                     krx[:, kt // 4,
                                         (kt % 4) * 128:(kt % 4 + 1) * 128],
                                     qrope[:, h, qof:NLOC], start=False, stop=True)
                    pt = p_att.tile([P, NLOC], F32R, tag="pt")
                    nc.scalar.activation(out=pt[:, :w], in_=st[:, :w], func=AF.Exp)
                    msb = m1_sb[:, kt, :] if slab1 else m2_sb[:, kt - 8, :]
                    nc.gpsimd.tensor_tensor(out=pt[:, :w], in0=pt[:, :w], in1=msb,
                                            op=AL.mult)
                    if slab1:
                        nc.tensor.matmul(denA[:], ones128r[:], pt[:, 0:256],
                                         start=(kt == 0), stop=(kt == 7))
                        for half in range(2):
                            nc.tensor.matmul(ols[half][:],
                                             kvtm[:, kt, half * 128:(half + 1) * 128],
                                             pt[:, 0:256], start=(kt == 0), stop=(kt == 7))
                        pB = pt[:, 256:NLOC]
                    else:
                        pB = pt[:, 0:256]
                    nc.tensor.matmul(denB[:], ones128r[:], pB,
                                     start=(kt == 0), stop=(kt == 15))
                    for half in range(2):
                        nc.tensor.matmul(ols[2 + half][:],
                                         kvtm[:, kt, half * 128:(half + 1) * 128],
                                         pB, start=(kt == 0), stop=(kt == 15))
                for s in range(2):
                    olsb = p_att.tile([P, 2, 256], F32R, tag="olsb")
                    for half in range(2):
                        nc.vector.tensor_copy(out=olsb[:, half, :],
                                              in_=ols[2 * s + half][:])
                    rb = p_att.tile([P, 256], F32, tag="rb")
                    nc.vector.reciprocal_approx_fast(out=rb[:], in_=dens[s][:])
                    yp = ps_st.tile([P, NLOC], F32, tag="st")
                    nc.tensor.matmul(yp[:, 0:256], wv_sb[:, 0, h * VD:(h + 1) * VD],
                                     olsb[:, 0, :], start=True, stop=False)
                    nc.tensor.matmul(yp[:, 0:256], wv_sb[:, 1, h * VD:(h + 1) * VD],
                                     olsb[:, 1, :], start=False, stop=True)
                    nc.vector.tensor_tensor(out=yT[:, h, s * 256:(s + 1) * 256],
                                            in0=yp[:, 0:256], in1=rb[:], op=AL.mult)

        g_m.close()
        g_q.close()
        g_kv.close()

        # ================= phase C: cproj + rms2 + gate =================
        p_wC2 = ES()
        p_sh = p_wC2.enter_context(tc.tile_pool(name="psh", bufs=1))
        sel8_sb = p_sh.tile([E, E * P], BF16, tag="sel8")
        nc.gpsimd.dma_start(out=sel8_sb[:], in_=sel8[:])
        sh1_sb = p_sh.tile([P, 8, INTER * NSH], FP8, tag="sh1")
        nc.gpsimd.dma_start(out=sh1_sb[:], in_=shw1[:])
        sh3_sb = p_sh.tile([P, 8, INTER * NSH], FP8, tag="sh3")
        nc.gpsimd.dma_start(out=sh3_sb[:], in_=shw3[:])
        sh2_sb = p_sh.tile([P, 8, D], BF16, tag="sh2")
        nc.gpsimd.dma_start(out=sh2_sb[:], in_=shw2[:])
        with tc.tile_pool(name="pscC", bufs=2) as p_scC, \
             tc.tile_pool(name="psmm", bufs=3, space="PSUM") as ps_mm, \
             tc.tile_pool(name="psrow2", bufs=1, space="PSUM") as ps_row2, \
             tc.tile_pool(name="pstp2", bufs=2, space="PSUM") as ps_tp2:

            nc.gpsimd.dma_start(out=x2[:], in_=xT.rearrange("(a p) n -> p a n", p=P))
            for ft in range(8):
                op = ps_mm.tile([P, NLOC], F32, tag="mm")
                for ds in range(8):
                    nc.tensor.matmul(op[:], cw_sb[:, ds, ft * 128:(ft + 1) * 128],
                                     yT[:, ds, :], start=(ds == 0), stop=(ds == 7))
                nc.vector.tensor_add(out=x2[:, ft, :], in0=op[:], in1=x2[:, ft, :])

            # rms2 (exact fp32 squares: feeds the gate)
            ss2 = ps_row2.tile([P, NLOC], F32, tag="row2")
            for ds in range(8):
                xsq = p_scC.tile([P, NLOC], F32R, tag="xsq32")
                nc.vector.tensor_mul(out=xsq[:], in0=x2[:, ds, :], in1=x2[:, ds, :])
                nc.tensor.matmul(ss2[:], ones128r[:], xsq[:],
                                 start=(ds == 0), stop=(ds == 7))
            # plain rsqrt for the gate (row 0 used by the transpose below)
            srow2 = p_scC.tile([P, NLOC], F32, tag="srow2")
            nc.scalar.activation(out=srow2[:], in_=ss2[:],
                                 func=AF.Sqrt, bias=eps128[:], scale=1.0 / D)
            nc.vector.reciprocal_approx_fast(out=srow2[:], in_=srow2[:])
            # HS-folded rsqrt for the fp8 activations
            s2b = p_scC.tile([P, NLOC], F32, tag="s2b")
            nc.scalar.activation(out=s2b[:], in_=ss2[:], func=AF.Sqrt,
                                 bias=eps128[:], scale=1.0 / (D * HS * HS))
            nc.vector.reciprocal_approx_fast(out=s2b[:], in_=s2b[:])
            for ds in range(8):
                nc.vector.tensor_tensor(out=h2f8[:, ds, :], in0=x2[:, ds, :],
                                        in1=s2b[:], op=AL.mult)

            # gate
            gp = ps_mm.tile([P, NLOC], F32, tag="mm")
            for ds in range(8):
                nc.tensor.matmul(gp[:E], gw_sb[:, ds, :], x2[:, ds, :],
                                 start=(ds == 0), stop=(ds == 7))
            g_sb = p_scC.tile([E, NLOC], F32, tag="gsb")
            nc.vector.tensor_copy(out=g_sb[:], in_=gp[:E])
            for q4 in range(4):
                tp = ps_tp2.tile([P, P], F32, tag="tp2")
                nc.tensor.transpose(tp[:, 0:E], g_sb[:, q4 * 128:(q4 + 1) * 128],
                                    identf[0:E, 0:E])
                gt = p_scC.tile([P, E], F32, tag="gt")
                nc.vector.tensor_copy(out=gt[:], in_=tp[:, 0:E])
                s2tp = ps_tp2.tile([P, P], F32, tag="tp2")
                nc.tensor.transpose(s2tp[:, 0:1], srow2[0:1, q4 * 128:(q4 + 1) * 128],
                                    identf[0:1, 0:1])
                s2c = p_scC.tile([P, 1], F32, tag="s2c")
                nc.vector.tensor_copy(out=s2c[:], in_=s2tp[:, 0:1])
                nc.vector.tensor_scalar_mul(out=gt[:], in0=gt[:], scalar1=s2c[:])
                mx = p_scC.tile([P, 4], F32, tag="mx")
                nc.vector.tensor_reduce(out=mx[:, 0:1], in_=gt[:],
                                        axis=mybir.AxisListType.X, op=AL.max)
                nc.vector.tensor_scalar_mul(out=mx[:, 1:2], in0=mx[:, 0:1], scalar1=-1.0)
                e8 = p_scC.tile([P, E], F32, tag="e8")
                nc.scalar.activation(out=e8[:], in_=gt[:], func=AF.Exp,
                                     bias=mx[:, 1:2], accum_out=mx[:, 2:3])
                nc.vector.reciprocal_approx_fast(out=mx[:, 3:4], in_=mx[:, 2:3])
                srt = p_scC.tile([P, E], F32, tag="srt")
                nc.vector.max(out=srt[:], in_=e8[:])
                cwq = p_scC.tile([P, E], F32, tag="cwq")
                nc.vector.tensor_scalar(out=cwq[:], in0=e8[:], scalar1=srt[:, 1:2],
                                        scalar2=None, op0=AL.is_ge)
                nc.vector.tensor_mul(out=cwq[:], in0=cwq[:], in1=e8[:])
                nc.vector.tensor_scalar_mul(out=cwq[:], in0=cwq[:], scalar1=mx[:, 3:4])
                tp2 = ps_tp2.tile([P, P], F32, tag="tp2")
                nc.tensor.transpose(tp2[0:E, 0:P], cwq[:], identf[:])
                nc.vector.tensor_copy(out=cwT[:, q4 * 128:(q4 + 1) * 128],
                                      in_=tp2[0:E, 0:P])

        g_yT.close()

        # ================= phase D: MoE (fp8 DoubleRow, 2-pass) =================
        with tc.tile_pool(name="pscD", bufs=2) as p_scD, \
             tc.tile_pool(name="pwe2", bufs=2) as p_we2, \
             tc.tile_pool(name="phsh", bufs=1) as p_hsh:

            hshs = []
            cwb_sh = p_scD.tile([P, NLOC], F32, tag="cwbsh")
            nc.vector.memset(cwb_sh[:], float(1.0 / QW))

            def mlp13(w1s, w3s, hsh8, nft, cwb):
                for ft in range(nft):
                    g1 = ps_g.tile([P, NLOC], F32, tag="g1")
                    g3 = ps_g.tile([P, NLOC], F32, tag="g3")
                    for dp in range(4):
                        nc.tensor.matmul(g1[:], w1s[:, 2 * dp:2 * dp + 2,
                                                    ft * 128:(ft + 1) * 128],
                                         h2f8[:, 2 * dp:2 * dp + 2, :],
                                         perf_mode=DR, start=(dp == 0), stop=(dp == 3))
                    for dp in range(4):
                        nc.tensor.matmul(g3[:], w3s[:, 2 * dp:2 * dp + 2,
                                                    ft * 128:(ft + 1) * 128],
                                         h2f8[:, 2 * dp:2 * dp + 2, :],
                                         perf_mode=DR, start=(dp == 0), stop=(dp == 3))
                    sl = p_scD.tile([P, NLOC], F32, tag="silu")
                    nc.scalar.activation(out=sl[:], in_=g1[:], func=AF.Silu,
                                         scale=float(1.0 / (HS * QW)))
                    tmp = p_scD.tile([P, NLOC], F32, tag="tmp")
                    nc.vector.tensor_mul(out=tmp[:], in0=sl[:], in1=g3[:])
                    nc.gpsimd.tensor_tensor(out=hsh8[:, ft, :], in0=tmp[:],
                                            in1=cwb[:], op=AL.mult)

            # pass 1: routed experts
            p1 = ES()
            p_we = p1.enter_context(tc.tile_pool(name="pwe", bufs=2))
            ps_g = p1.enter_context(tc.tile_pool(name="psg", bufs=2, space="PSUM"))
            ps_bcd = p1.enter_context(tc.tile_pool(name="psbcd", bufs=2, space="PSUM"))
            # pass 1a: shared expert first (cw = 1; no gate dependency)
            hsh_sh = p_hsh.tile([P, 8, NLOC], BF16, tag="hshsh")
            mlp13(sh1_sb, sh3_sb, hsh_sh, 8, cwb_sh)
            for e in range(E):
                e1_sb = p_we.tile([P, 8, INTER], FP8, tag="we1")
                nc.sync.dma_start(out=e1_sb[:], in_=ew1[e])
                e3_sb = p_we.tile([P, 8, INTER], FP8, tag="we3")
                nc.sync.dma_start(out=e3_sb[:], in_=ew3[e])
                hsh8 = p_hsh.tile([P, 4, NLOC], BF16, tag=f"hsh{e}")
                hshs.append(hsh8)
                cwp = ps_bcd.tile([P, NLOC], F32, tag="bc")
                nc.tensor.matmul(cwp[:], sel8_sb[:, e * P:(e + 1) * P], cwT[:],
                                 start=True, stop=True)
                cwb = p_scD.tile([P, NLOC], F32, tag="cwb")
                nc.scalar.activation(out=cwb[:], in_=cwp[:], func=AF.Copy)
                mlp13(e1_sb, e3_sb, hsh8, 4, cwb)

            p1.close()
            # pass 2: w2 accumulation across experts + shared, then residual
            with tc.tile_pool(name="psacc", bufs=1, space="PSUM") as ps_acc:
                acc = ps_acc.tile([P, 8, NLOC], F32, tag="acc")
                for e in range(E):
                    e2_sb = p_we2.tile([P, 4, D], BF16, tag="we2")
                    nc.sync.dma_start(out=e2_sb[:], in_=ew2[e])
                    for ft in range(8):
                        for ds in range(4):
                            nc.tensor.matmul(acc[:, ft, :],
                                             e2_sb[:, ds, ft * 128:(ft + 1) * 128],
                                             hshs[e][:, ds, :],
                                             start=(e == 0 and ds == 0), stop=False)
                for ft in range(8):
                    for ds in range(8):
                        nc.tensor.matmul(acc[:, ft, :],
                                         sh2_sb[:, ds, ft * 128:(ft + 1) * 128],
                                         hsh_sh[:, ds, :],
                                         start=False, stop=(ds == 7))
                for ft in range(8):
                    msb = p_scD.tile([P, NLOC], F32, tag="msb")
                    nc.scalar.activation(out=msb[:], in_=acc[:, ft, :], func=AF.Copy,
                                         scale=float(1.0 / (HS * QW)))
                    nc.vector.tensor_add(out=x2[:, ft, :], in0=msb[:], in1=x2[:, ft, :])
                    nc.sync.dma_start(out=out_xT[:, ft, :], in_=x2[:, ft, :])

        p_wC2.close()
        p_wC1.close()
        g_x.close()

    nc.finalize()
    return nc


# ============================ host side ============================

_CACHE = {}


def _prep_shared(inputs):
    perm = _rope_perm()
    f = np.float32
    latent_w = (np.asarray(inputs["latent_w"], f)
                * np.asarray(inputs["rmsn1_w"], f)[:, None]).copy()
    latent_w[:, QLR + KVLR:] = latent_w[:, QLR + KVLR:][:, perm]
    q_up = (np.asarray(inputs["q_up_w"], f)
            * np.asarray(inputs["q_norm_w"], f)[:, None]).copy()
    for h in range(H):
        c0 = h * QKD + NOPE
        q_up[:, c0:c0 + ROPE] = q_up[:, c0:c0 + ROPE][:, perm]
    q_up *= SCALE
    q_up_pad = np.zeros((QLR, H * 256), f)
    for h in range(H):
        q_up_pad[:, h * 256:h * 256 + QKD] = q_up[:, h * QKD:(h + 1) * QKD]
    kv_up = (np.asarray(inputs["kv_up_w"], f)
             * np.asarray(inputs["kv_norm_w"], f)[:, None])
    wk = np.stack([kv_up[:, h * (NOPE + VD):h * (NOPE + VD) + NOPE].T
                   for h in range(H)], axis=1)   # [NOPE, H, KVLR]
    wv = np.concatenate([kv_up[:, h * (NOPE + VD) + NOPE:(h + 1) * (NOPE + VD)]
                         for h in range(H)], axis=1)
    r2 = np.asarray(inputs["rmsn2_w"], f)[:, None]
    latw_t = _tile_w(latent_w)           # [P, 8, 704]
    shared = {
        "ident": np.eye(P, dtype=f),
        "latq_w": np.ascontiguousarray(latw_t[:, :, :QLR]).astype(BFNP),
        "latkv_w": np.concatenate(
            [latw_t[:, :, QLR:],
             np.zeros((P, 8, 128 - ROPE), np.float32)], axis=2).astype(BFNP),
        "q_up": _tile_w(q_up_pad),
        "wkTT": np.ascontiguousarray(wk.astype(f)),
        "wv_w": _tile_w(wv.astype(f)),
        "cproj_w": _tile_w(np.asarray(inputs["c_proj_w"], f)),
        "gate_w": _tile_w(np.asarray(inputs["gate_w"], f) * r2),
        "shw1": _f8(_tile_w(np.asarray(inputs["sh_w1"], f) * r2) * QW),
        "shw3": _f8(_tile_w(np.asarray(inputs["sh_w3"], f) * r2) * QW),
        "shw2": (_tile_w(np.asarray(inputs["sh_w2"], f)) * QW).astype(BFNP),
        "ew1": np.stack([_f8(_tile_w(np.asarray(inputs["e_w1"], f)[e] * r2) * QW)
                         for e in range(E)]),
        "ew3": np.stack([_f8(_tile_w(np.asarray(inputs["e_w3"], f)[e] * r2) * QW)
                         for e in range(E)]),
        "ew2": np.stack([(_tile_w(np.asarray(inputs["e_w2"], f)[e]) * QW).astype(BFNP)
                         for e in range(E)]),
        "sel8": np.repeat(np.eye(E, dtype=f) * (1.0 / QW), P, axis=1)
                  .reshape(E, E * P).astype(BFNP),
    }
    return shared


def _prep_core(inputs, c):
    f = np.float32
    pos = _core_positions(c)
    b = c // 4
    gidx = b * T + pos
    xflat = np.asarray(inputs["x"], dtype=f).reshape(N, D)
    xT_c = np.ascontiguousarray(xflat[gidx].T)
    xTb_c = np.ascontiguousarray(
        xflat[gidx].reshape(NLOC, 8, P).transpose(2, 1, 0)).astype(BFNP)
    xb_c = np.ascontiguousarray(
        xflat[b * T:(b + 1) * T].reshape(T, 8, P).transpose(2, 1, 0)
    ).astype(BFNP)
    cosb = np.ascontiguousarray(np.asarray(inputs["freqs_cos"], f).T
                                ).reshape(ROPE // 2, 4, 512).astype(BFNP)
    sinb = np.ascontiguousarray(np.asarray(inputs["freqs_sin"], f).T
                                ).reshape(ROPE // 2, 4, 512).astype(BFNP)
    cosT = np.asarray(inputs["freqs_cos"], f)[pos].T   # [32, NLOC]
    sinT = np.asarray(inputs["freqs_sin"], f)[pos].T
    cosT4 = np.ascontiguousarray(
        np.repeat(cosT[:, None, :], 4, axis=1)).astype(BFNP)
    sinT4 = np.ascontiguousarray(
        np.repeat(sinT[:, None, :], 4, axis=1)).astype(BFNP)
    k_abs = (np.arange(8)[:, None] * 128 + np.arange(P)[None, :])
    m1 = np.where(k_abs[:, :, None] <= pos[None, None, :], 1.0, 0.0)
    k_abs2 = ((np.arange(8, 16))[:, None] * 128 + np.arange(P)[None, :])
    m2 = np.where(k_abs2[:, :, None] <= pos[None, None, 256:], 1.0, 0.0)
    return {
        "xT": xT_c, "xTb": xTb_c, "xb": xb_c, "cosbT": cosb, "sinbT": sinb,
        "cosT4": cosT4, "sinT4": sinT4,
        "mask1": np.ascontiguousarray(m1.transpose(1, 0, 2)).astype(BFNP),
        "mask2": np.ascontiguousarray(m2.transpose(1, 0, 2)).astype(BFNP),
    }, gidx


def run(inputs, trace=False, **kw):
    if "nc" not in _CACHE:
        _CACHE["nc"] = build()
    nc = _CACHE["nc"]
    shared = _prep_shared(inputs)
    in_maps = []
    gidxs = []
    for c in range(NCORES):
        m, gidx = _prep_core(inputs, c)
        m.update(shared)
        in_maps.append(m)
        gidxs.append(gidx)
    res = run_bass_kernel_spmd(nc, in_maps, core_ids=list(range(NCORES)),
                               trace=trace, **kw)
    full = np.empty((N, D), dtype=np.float32)
    for c in range(NCORES):
        full[gidxs[c]] = np.asarray(res.results[c]["out_xT"], np.float32
                                    ).transpose(2, 1, 0).reshape(NLOC, D)
    return full.reshape(B, T, D), res


def kernel(**inputs):
    out, _ = run(inputs)
    return out
